# revision 1
# baseline (speedup 1.0000x reference)
import os, sys
import numpy as np

sys.path.insert(0, "/opt/trn_rl_repo")

from concourse import bass, bacc, bass_utils
from concourse import mybir
from concourse.tile import TileContext

F32 = mybir.dt.float32
F32R = mybir.dt.float32r
F16 = mybir.dt.float16
ALU = mybir.AluOpType
ACTF = mybir.ActivationFunctionType

A = 32          # in_maps
B = 32          # out_maps
C = 16          # atoms
H = 64
W = 64
NCORES = 8
ROWS = H // NCORES
NPOS = ROWS * W             # 512 positions per core
NCHUNK = 128
NCH = NPOS // NCHUNK        # 4 chunks
CB = C * B                  # 512
EPS = 1e-4
G = 14                      # a-values packed per partition-stride group
J = 3                       # ceil(A/G)
NP_IN = 9 * G               # 126 partitions for patch/weight tensors

_CACHE = {}


def _build_nc(num_routes: int):
    nc = bacc.Bacc(None, target_bir_lowering=False)

    patches_h_d = nc.declare_dram_parameter("patches_h", [9, A, NPOS], F32R, isOutput=False)
    patches_l_d = nc.declare_dram_parameter("patches_l", [9, A, NPOS], F32R, isOutput=False)
    wv_h_d = nc.declare_dram_parameter("wv_h", [9, A, CB], F32R, isOutput=False)
    wv_l_d = nc.declare_dram_parameter("wv_l", [9, A, CB], F32R, isOutput=False)
    out_d = nc.declare_dram_parameter("out", [NPOS, CB], F32, isOutput=True)

    with TileContext(nc) as tc:
        with (
            tc.tile_pool(name="const", bufs=1) as cpool,
            tc.tile_pool(name="psum", bufs=6, space="PSUM") as pp,
            tc.tile_pool(name="p1psum", bufs=2, space="PSUM") as pp1,
            tc.tile_pool(name="v1", bufs=1) as v1pool,
            tc.tile_pool(name="v2", bufs=1) as v2pool,
            tc.tile_pool(name="work", bufs=1) as wk,
            tc.tile_pool(name="small", bufs=1) as sm,
        ):

            for k in range(NCH):
                v1 = v1pool.tile([NCHUNK, CB * A], F16)    # [p, (c b a)]
                v2 = v2pool.tile([NCHUNK, CB * A], F32)    # [p, (b a c)]
                p1raw = sm.tile([NCHUNK, CB], F32, tag="p1raw")
                p1ps = pp1.tile([NCHUNK, CB], F32)
                v14 = v1[:].rearrange("p (c b a) -> p c b a", c=C, b=B)
                v24 = v2[:].rearrange("p (b a c) -> p b a c", b=B, a=A)

                pch = cpool.tile([9, A * NCHUNK], F32R, tag="pch")
                pcl = cpool.tile([9, A * NCHUNK], F32R, tag="pcl")
                ksl = slice(k * NCHUNK, (k + 1) * NCHUNK)
                nc.sync.dma_start(out=pch[:].rearrange("q (a n) -> q a n", a=A),
                                  in_=patches_h_d.ap()[:, :, ksl])
                nc.sync.dma_start(out=pcl[:].rearrange("q (a n) -> q a n", a=A),
                                  in_=patches_l_d.ap()[:, :, ksl])
                pch3 = pch[:].rearrange("q (a n) -> q a n", a=A)
                pcl3 = pcl[:].rearrange("q (a n) -> q a n", a=A)
                QA = 8
                for qa in range(A // QA):
                    wqh = cpool.tile([9, QA * CB], F32R, tag="wqh")
                    wql = cpool.tile([9, QA * CB], F32R, tag="wql")
                    asl = slice(qa * QA, (qa + 1) * QA)
                    nc.sync.dma_start(out=wqh[:].rearrange("q (a n) -> q a n", a=QA),
                                      in_=wv_h_d.ap()[:, asl, :])
                    nc.sync.dma_start(out=wql[:].rearrange("q (a n) -> q a n", a=QA),
                                      in_=wv_l_d.ap()[:, asl, :])
                    wqh3 = wqh[:].rearrange("q (a n) -> q a n", a=QA)
                    wql3 = wql[:].rearrange("q (a n) -> q a n", a=QA)
                    for al in range(QA):
                        a = qa * QA + al
                        hi_p = pch3[:, a, :]
                        lo_p = pcl3[:, a, :]
                        hi_w = wqh3[:, al, :]
                        lo_w = wql3[:, al, :]
                        nc.tensor.matmul(out=p1ps[:], lhsT=hi_p, rhs=hi_w,
                                         start=(a == 0), stop=False)
                        nc.tensor.matmul(out=p1ps[:], lhsT=hi_p, rhs=lo_w,
                                         start=False, stop=False)
                        nc.tensor.matmul(out=p1ps[:], lhsT=lo_p, rhs=hi_w,
                                         start=False, stop=(a == A - 1))
                        vps = pp.tile([NCHUNK, CB], F32)
                        nc.tensor.matmul(out=vps[:], lhsT=hi_p, rhs=hi_w,
                                         start=True, stop=False)
                        nc.tensor.matmul(out=vps[:], lhsT=hi_p, rhs=lo_w,
                                         start=False, stop=False)
                        nc.tensor.matmul(out=vps[:], lhsT=lo_p, rhs=hi_w,
                                         start=False, stop=True)
                        srcv = vps[:].rearrange("p (c b) -> p c b", c=C)
                        nc.vector.tensor_copy(out=v24[:, :, a, :],
                                              in_=srcv.rearrange("p c b -> p b c"))
                        nc.gpsimd.tensor_copy(
                            out=v14[:, :, :, a],
                            in_=v24[:, :, a, :].rearrange("p b c -> p c b"))
                nc.vector.tensor_scalar_mul(out=p1raw[:], in0=p1ps[:], scalar1=1.0 / A)

                logits = sm.tile([NCHUNK, B * A], F32, tag="logits")   # [p, (b a)]
                lsm = sm.tile([NCHUNK, B * A], F32, tag="lsm")
                lsm16 = wk.tile([NCHUNK, B * A], F16, tag="lsm16")
                pcur = sm.tile([NCHUNK, CB], F32, tag="pcur")          # [p, (c b)]
                praw = sm.tile([NCHUNK, CB], F32, tag="praw")
                tmpf = wk.tile([NCHUNK, CB * A // 4], F32, tag="tmpf")
                tmph = tmpf[:].bitcast(F16)

                def squash(p_raw):
                    sq = sm.tile([NCHUNK, B], F32, tag="sq")
                    den = sm.tile([NCHUNK, B], F32, tag="den")
                    fac = sm.tile([NCHUNK, B], F32, tag="fac")
                    p2t = wk.tile([NCHUNK, CB], F32, tag="p2")
                    p2 = p2t[:]
                    nc.vector.tensor_tensor(out=p2, in0=p_raw[:], in1=p_raw[:],
                                            op=ALU.mult)
                    nc.vector.tensor_reduce(
                        out=sq[:],
                        in_=p2.rearrange("p (c b) -> p b c", c=C),
                        axis=mybir.AxisListType.X, op=ALU.add)
                    nc.vector.tensor_scalar_add(out=sq[:], in0=sq[:], scalar1=EPS)
                    nc.scalar.activation(out=den[:], in_=sq[:], func=ACTF.Sqrt)
                    nc.vector.tensor_scalar_add(out=fac[:], in0=sq[:], scalar1=1.0)
                    nc.vector.tensor_tensor(out=den[:], in0=den[:], in1=fac[:],
                                            op=ALU.mult)
                    nc.vector.tensor_scalar_add(out=den[:], in0=den[:], scalar1=EPS)
                    nc.vector.reciprocal(out=den[:], in_=den[:])
                    nc.vector.tensor_tensor(out=fac[:], in0=sq[:], in1=den[:],
                                            op=ALU.mult)
                    nc.vector.tensor_tensor(
                        out=pcur[:].rearrange("p (c b) -> p c b", c=C),
                        in0=p_raw[:].rearrange("p (c b) -> p c b", c=C),
                        in1=fac[:].unsqueeze(1).to_broadcast([NCHUNK, C, B]),
                        op=ALU.mult)

                def delta_update(first):
                    # tmpf[p,b,a,c] = V2 * pcur (bcast a); reduce c -> delta [p,(b a)]
                    QB = B // 4
                    pc_bac = pcur[:].rearrange("p (c b) -> p b c", c=C)
                    for h in range(4):
                        bs = slice(h * QB, (h + 1) * QB)
                        nc.vector.tensor_tensor(
                            out=tmpf[:].rearrange("p (b a c) -> p b a c", b=QB, a=A),
                            in0=v24[:, bs, :, :],
                            in1=pc_bac[:, bs, :].unsqueeze(2)
                                .to_broadcast([NCHUNK, QB, A, C]),
                            op=ALU.mult)
                        nc.vector.tensor_reduce(
                            out=logits[:].rearrange("p (b a) -> p b a", b=B)[:, bs, :],
                            in_=tmpf[:].rearrange("p (b a c) -> p b a c", b=QB, a=A),
                            axis=mybir.AxisListType.X, op=ALU.add)
                    if first:
                        nc.vector.tensor_scalar_add(out=logits[:], in0=logits[:],
                                                    scalar1=1.0 / A)
                    else:
                        nc.vector.tensor_tensor(out=logits[:], in0=logits[:],
                                                in1=lsm[:], op=ALU.add)

                def softmax():
                    ssum = sm.tile([NCHUNK, B], F32, tag="ssum")
                    nc.scalar.activation(out=lsm[:], in_=logits[:], func=ACTF.Exp)
                    nc.vector.tensor_reduce(
                        out=ssum[:], in_=lsm[:].rearrange("p (b a) -> p b a", b=B),
                        axis=mybir.AxisListType.X, op=ALU.add)
                    nc.vector.reciprocal(out=ssum[:], in_=ssum[:])
                    nc.vector.tensor_tensor(
                        out=lsm[:].rearrange("p (b a) -> p b a", b=B),
                        in0=lsm[:].rearrange("p (b a) -> p b a", b=B),
                        in1=ssum[:].unsqueeze(2).to_broadcast([NCHUNK, B, A]),
                        op=ALU.mult)
                    nc.gpsimd.tensor_copy(out=lsm16[:], in_=lsm[:])

                def preds_from_lsm():
                    HC = C // 2
                    for h in range(2):
                        cs = slice(h * HC, (h + 1) * HC)
                        nc.vector.tensor_tensor(
                            out=tmph.rearrange("p (c b a) -> p c b a", c=HC, b=B),
                            in0=v14[:, cs, :, :],
                            in1=lsm16[:].rearrange("p (b a) -> p b a", b=B)
                                .unsqueeze(1).to_broadcast([NCHUNK, HC, B, A]),
                            op=ALU.mult)
                        nc.vector.tensor_reduce(
                            out=praw[:].rearrange("p (c b) -> p c b", c=C)[:, cs, :],
                            in_=tmph.rearrange("p (c b a) -> p c b a", c=HC, b=B),
                            axis=mybir.AxisListType.X, op=ALU.add)

                squash(p1raw)
                if num_routes >= 2:
                    delta_update(True)
                    for it in range(2, num_routes + 1):
                        softmax()
                        preds_from_lsm()
                        squash(praw)
                        if it < num_routes:
                            delta_update(False)

                nc.sync.dma_start(out=out_d.ap()[k * NCHUNK:(k + 1) * NCHUNK, :],
                                  in_=pcur[:])

    nc.compile()
    return nc


def kernel(x=None, weights=None, num_routes=3, **kw):
    x = np.asarray(x, dtype=np.float32)
    weights = np.asarray(weights, dtype=np.float32)
    nr = int(num_routes)

    if nr not in _CACHE:
        _CACHE[nr] = _build_nc(nr)
    nc = _CACHE[nr]

    xp = np.zeros((A, H + 2, W + 2), dtype=np.float32)
    xp[:, 1:-1, 1:-1] = x

    f16r = lambda t: t.astype(np.float16).astype(np.float32)
    wvf = np.ascontiguousarray(weights.reshape(9, A, CB))
    wv_hi = f16r(wvf)
    wv_lo = wvf - wv_hi

    in_maps = []
    for core in range(NCORES):
        r0 = core * ROWS
        pat = np.empty((9, A, ROWS, W), dtype=np.float32)
        for dp in range(3):
            for dq in range(3):
                pat[dp * 3 + dq] = xp[:, r0 + dp:r0 + dp + ROWS, dq:dq + W]
        patf = np.ascontiguousarray(pat.reshape(9, A, NPOS))
        pat_hi = f16r(patf)
        in_maps.append({"patches_h": pat_hi, "patches_l": patf - pat_hi,
                        "wv_h": wv_hi, "wv_l": wv_lo})

    res = bass_utils.run_bass_kernel_spmd(nc, in_maps, core_ids=list(range(NCORES)))

    out = np.empty((B, C, H, W), dtype=np.float32)
    for core in range(NCORES):
        o = np.asarray(res.results[core]["out"]).reshape(ROWS, W, C, B)
        out[:, :, core * ROWS:(core + 1) * ROWS, :] = o.transpose(3, 2, 0, 1)
    return out


def profile_once(inputs):
    """Run once with NTFF tracing on core 0 and return HW exec time in ns."""
    x = np.asarray(inputs["x"], dtype=np.float32)
    weights = np.asarray(inputs["weights"], dtype=np.float32)
    nr = int(inputs.get("num_routes", 3))
    if nr not in _CACHE:
        _CACHE[nr] = _build_nc(nr)
    nc = _CACHE[nr]
    xp = np.zeros((A, H + 2, W + 2), dtype=np.float32)
    xp[:, 1:-1, 1:-1] = x
    f16r = lambda t: t.astype(np.float16).astype(np.float32)
    wvf = np.ascontiguousarray(weights.reshape(9, A, CB))
    wv_hi = f16r(wvf); wv_lo = wvf - wv_hi
    in_maps = []
    for core in range(NCORES):
        r0 = core * ROWS
        pat = np.empty((9, A, ROWS, W), dtype=np.float32)
        for dp in range(3):
            for dq in range(3):
                pat[dp * 3 + dq] = xp[:, r0 + dp:r0 + dp + ROWS, dq:dq + W]
        patf = np.ascontiguousarray(pat.reshape(9, A, NPOS))
        pat_hi = f16r(patf)
        in_maps.append({"patches_h": pat_hi, "patches_l": patf - pat_hi,
                        "wv_h": wv_hi, "wv_l": wv_lo})
    res = bass_utils.run_bass_kernel_spmd(nc, in_maps,
                                          core_ids=list(range(NCORES)),
                                          trace=True, trace_cores=[0])
    if res.exec_time_ns is not None:
        return int(res.exec_time_ns)
    raise RuntimeError("no exec_time_ns from trace")



# revision 3
# speedup vs baseline: 2.1299x; 2.1299x over previous
import os, sys
import numpy as np

sys.path.insert(0, "/opt/trn_rl_repo")

from concourse import bass, bacc, bass_utils
from concourse import mybir
from concourse.tile import TileContext

F32 = mybir.dt.float32
F16 = mybir.dt.float16
ALU = mybir.AluOpType
ACTF = mybir.ActivationFunctionType

A = 32          # in_maps
B = 32          # out_maps
C = 16          # atoms
H = 64
W = 64
NCORES = 8
ROWS = H // NCORES
NPOS = ROWS * W             # 512 positions per core
NCHUNK = 128
NCH = NPOS // NCHUNK        # 4 chunks
CB = C * B                  # 512, (c,b)-order: idx = c*B + b
EPS = 1e-4
AG = 4                      # a-group size for PSUM staging
NG = A // AG                # 8 groups
P = NCHUNK

# stage split (DVE : Pool)
BSPL = 16                   # delta2 b-split: DVE gets b[0:BSPL]
CSPL = 10                   # preds c-split: DVE gets c[0:CSPL]
GSPL = 5                    # delta1 group split: DVE gets groups [0:GSPL]

_CACHE = {}


def _build_nc(num_routes: int):
    nc = bacc.Bacc(None, target_bir_lowering=False)

    pat2_d = nc.declare_dram_parameter("pat2", [9, A, 2, NPOS], F16, isOutput=False)
    wvh_d = nc.declare_dram_parameter("wvh", [9, A, CB], F16, isOutput=False)
    wvl_d = nc.declare_dram_parameter("wvl", [9, A, CB], F16, isOutput=False)
    pp1_d = nc.declare_dram_parameter("pp1", [96, 2, 3, NPOS], F16, isOutput=False)
    wp1_d = nc.declare_dram_parameter("wp1", [96, 2, 3, CB], F16, isOutput=False)
    out_d = nc.declare_dram_parameter("out", [NPOS, CB], F32, isOutput=True)

    with TileContext(nc) as tc:
        with (
            tc.tile_pool(name="wp1", bufs=1) as wp1pool,
            tc.tile_pool(name="win", bufs=2) as winpool,
            tc.tile_pool(name="v16", bufs=2) as v16pool,
            tc.tile_pool(name="v32", bufs=2) as v32pool,
            tc.tile_pool(name="psv", bufs=2, space="PSUM") as psv,
            tc.tile_pool(name="rt", bufs=1) as rt,
        ):
            # ---- static p1 weights [96, (hl q n)] one DMA
            wp1t = wp1pool.tile([96, 2 * 3 * CB], F16, name="wp1t", tag="wp1t")
            nc.sync.dma_start(out=wp1t[:].rearrange("p (h q n) -> p h q n", h=2, q=3),
                              in_=wp1_d.ap())
            wp1v = wp1t[:].rearrange("p (h q n) -> p h q n", h=2, q=3)

            # ---- resident hi weights [9, A*CB] f16
            wvh = wp1pool.tile([9, A * CB], F16, name="wvh", tag="wvh")
            for qw in range(4):
                asl = slice(qw * 8, (qw + 1) * 8)
                nc.sync.dma_start(
                    out=wvh[:].rearrange("q (a n) -> q a n", a=A)[:, asl, :],
                    in_=wvh_d.ap()[:, asl, :])
            wvh3 = wvh[:].rearrange("q (a n) -> q a n", a=A)

            # ---- shared routing scratch (aliased: delta1 f32 views live in
            # scrA/scrB/tr2/tr3 whose f16 users are temporally disjoint)
            scrA = rt.tile([P, 16384], F16, tag="scrA")   # prod16 | pd1v/pd1g
            scrB = rt.tile([P, 8192], F16, tag="scrB")    # tree1  | t1d1v/g | praw/t3d1
            tr2 = rt.tile([P, 4096], F16, tag="tr2")      # tree2  | t2d1v/g | elog
            tr3 = rt.tile([P, 2048], F16, tag="tr3")      # tree3
            tr4 = rt.tile([P, 1024], F16, tag="tr4")      # tree4
            pcur1t = rt.tile([P, CB], F32, tag="pcur1t")
            SCR = {
                "prod16": scrA[:],
                "tree1": scrB[:],
                "tree2": tr2[:],
                "tree3": tr3[:],
                "tree4": tr4[:],
                "sqp2": tr4[:, 0:1024].bitcast(F32),
                "pd1v": scrA[:, 0:4096].bitcast(F32),
                "pd1g": scrA[:, 4096:8192].bitcast(F32),
                "t1d1v": scrB[:, 0:2048].bitcast(F32),
                "t1d1g": scrB[:, 2048:4096].bitcast(F32),
                "t2d1v": tr2[:, 0:1024].bitcast(F32),
                "t2d1g": tr2[:, 1024:2048].bitcast(F32),
                "t3d1v": scrB[:, 5120:5632].bitcast(F32),
                "t3d1g": scrB[:, 5632:6144].bitcast(F32),
                "elog": tr2[:, 2048:4096].bitcast(F32),
                "praw": scrB[:, 4096:5120].bitcast(F32),
                "pcur1": pcur1t[:],
            }

            def emit_votes(k):
                """DMA + PE voting + Act drains for chunk k."""
                ksl = slice(k * NCHUNK, (k + 1) * NCHUNK)
                pp1t = winpool.tile([96, 2 * 3 * NCHUNK], F16, name="pp1t", tag="pp1t", bufs=1)
                nc.sync.dma_start(
                    out=pp1t[:].rearrange("p (h q n) -> p h q n", h=2, q=3),
                    in_=pp1_d.ap()[:, :, :, ksl])
                pp1v = pp1t[:].rearrange("p (h q n) -> p h q n", h=2, q=3)
                # p1 matmuls into a shared psv-slot (uses first bank only)
                p1t = psv.tile([P, AG * CB], F32, tag="vps")
                p1ps = p1t[:, :CB]
                for q in range(3):
                    nc.tensor.matmul(out=p1ps, lhsT=pp1v[:, 0, q, :], rhs=wp1v[:, 0, q, :],
                                     start=(q == 0), stop=False)
                    nc.tensor.matmul(out=p1ps, lhsT=pp1v[:, 0, q, :], rhs=wp1v[:, 1, q, :],
                                     start=False, stop=False)
                    nc.tensor.matmul(out=p1ps, lhsT=pp1v[:, 1, q, :], rhs=wp1v[:, 0, q, :],
                                     start=False, stop=(q == 2))
                p1raw = rt.tile([P, CB], F32, tag="p1raw")
                nc.scalar.mul(out=p1raw[:], in_=p1ps, mul=1.0 / A)

                v16b = v16pool.tile([P, CB * A], F16, tag="v16b")   # (c,b,a)
                v16b4 = v16b[:].rearrange("p (c b a) -> p c b a", c=C, b=B)
                v32gs = []
                for g in range(NG):
                    w0 = g * AG
                    pkt = winpool.tile([9, AG * 2 * NCHUNK], F16, name="pk", tag="pk")
                    nc.sync.dma_start(
                        out=pkt[:].rearrange("q (a h n) -> q a h n", a=AG, h=2),
                        in_=pat2_d.ap()[:, w0:w0 + AG, :, ksl])
                    pkv = pkt[:].rearrange("q (a h n) -> q a h n", a=AG, h=2)
                    if True:
                        wlt = winpool.tile([9, AG * CB], F16, name="wgl", tag="wgl")
                        nc.sync.dma_start(
                            out=wlt[:].rearrange("q (a n) -> q a n", a=AG),
                            in_=wvl_d.ap()[:, w0:w0 + AG, :])
                        wl3 = wlt[:].rearrange("q (a n) -> q a n", a=AG)
                    vps = psv.tile([P, AG * CB], F32, tag="vps")
                    for ai in range(AG):
                        a_glob = g * AG + ai
                        aw = ai
                        osl = vps[:, ai * CB:(ai + 1) * CB]
                        nc.tensor.matmul(out=osl, lhsT=pkv[:, ai, 0, :], rhs=wvh3[:, a_glob, :],
                                         start=True, stop=False)
                        nc.tensor.matmul(out=osl, lhsT=pkv[:, ai, 0, :], rhs=wl3[:, aw, :],
                                         start=False, stop=False)
                        nc.tensor.matmul(out=osl, lhsT=pkv[:, ai, 1, :], rhs=wvh3[:, a_glob, :],
                                         start=False, stop=True)
                    for ai in range(AG):
                        a_glob = g * AG + ai
                        nc.scalar.copy(
                            out=v16b4[:, :, :, a_glob],
                            in_=vps[:, ai * CB:(ai + 1) * CB].rearrange("p (c b) -> p c b", c=C))
                    v32g = v32pool.tile([P, AG * CB], F32, tag="v32g")
                    nc.scalar.copy(out=v32g[:], in_=vps[:])
                    v32gs.append(v32g)
                return p1raw, v16b4, v32gs

            def squash(praw_cb, pcur_tag, sq_tag):
                p2 = SCR["sqp2"]
                nc.scalar.activation(out=p2, in_=praw_cb, func=ACTF.Square)
                s = rt.tile([P, B], F32, tag=f"{sq_tag}s")
                nc.vector.tensor_reduce(
                    out=s[:], in_=p2.rearrange("p (c b) -> p b c", c=C),
                    axis=mybir.AxisListType.X, op=ALU.add)
                nc.vector.tensor_scalar_add(out=s[:], in0=s[:], scalar1=EPS)  # s = sq
                nrm = rt.tile([P, B], F32, tag=f"{sq_tag}n")
                nc.scalar.activation(out=nrm[:], in_=s[:], func=ACTF.Sqrt)
                fac = rt.tile([P, B], F32, tag=f"{sq_tag}f")
                nc.vector.scalar_tensor_tensor(out=fac[:], in0=s[:], scalar=1.0,
                                               in1=nrm[:], op0=ALU.add, op1=ALU.mult)
                nc.vector.tensor_scalar_add(out=fac[:], in0=fac[:], scalar1=EPS)
                nc.vector.reciprocal(out=fac[:], in_=fac[:])
                nc.vector.tensor_tensor(out=fac[:], in0=s[:], in1=fac[:], op=ALU.mult)
                if pcur_tag == "pcur1":
                    pcur_ap = SCR["pcur1"]
                else:
                    pcur_t = rt.tile([P, CB], F32, tag=pcur_tag, name=pcur_tag)
                    pcur_ap = pcur_t[:]
                nc.vector.tensor_tensor(
                    out=pcur_ap.rearrange("p (c b) -> p c b", c=C),
                    in0=praw_cb.rearrange("p (c b) -> p c b", c=C),
                    in1=fac[:].unsqueeze(1).to_broadcast([P, C, B]),
                    op=ALU.mult)
                return pcur_ap

            def emit_delta1(v32gs, pcur1, logits1, groups):
                """logits1 (a,b) f32 = sum_c V*P1 (groups subset)."""
                l1v = logits1[:].rearrange("p (a b) -> p a b", a=A)
                p1b = pcur1.rearrange("p (c b) -> p c b", c=C) \
                    .unsqueeze(1).to_broadcast([P, AG, C, B])
                for g in groups:
                    eng = nc.vector if g < GSPL else nc.gpsimd
                    sfx = "v" if g < GSPL else "g"
                    v32g4 = v32gs[g][:].rearrange("p (a c b) -> p a c b", a=AG, c=C)
                    pd4 = SCR[f"pd1{sfx}"].rearrange("p (a c b) -> p a c b", a=AG, c=C)
                    eng.tensor_tensor(out=pd4, in0=v32g4, in1=p1b, op=ALU.mult)
                    t14 = SCR[f"t1d1{sfx}"].rearrange("p (a c b) -> p a c b", a=AG, c=8)
                    eng.tensor_tensor(out=t14, in0=pd4[:, :, 0:8, :], in1=pd4[:, :, 8:16, :], op=ALU.add)
                    t24 = SCR[f"t2d1{sfx}"].rearrange("p (a c b) -> p a c b", a=AG, c=4)
                    eng.tensor_tensor(out=t24, in0=t14[:, :, 0:4, :], in1=t14[:, :, 4:8, :], op=ALU.add)
                    t34 = SCR[f"t3d1{sfx}"].rearrange("p (a c b) -> p a c b", a=AG, c=2)
                    eng.tensor_tensor(out=t34, in0=t24[:, :, 0:2, :], in1=t24[:, :, 2:4, :], op=ALU.add)
                    eng.tensor_tensor(out=l1v[:, g * AG:(g + 1) * AG, :],
                                      in0=t34[:, :, 0, :], in1=t34[:, :, 1, :], op=ALU.add)

            def softmax(logits, layout, lsm16):
                elog = SCR["elog"]
                nc.scalar.activation(out=elog, in_=logits[:], func=ACTF.Exp)
                ssum = rt.tile([P, B], F32, tag="sqs")
                if layout == "ab":
                    ev = elog.rearrange("p (a b) -> p a b", a=A)
                    nc.vector.tensor_reduce(out=ssum[:], in_=ev.rearrange("p a b -> p b a"),
                                            axis=mybir.AxisListType.X, op=ALU.add)
                    nc.vector.reciprocal(out=ssum[:], in_=ssum[:])
                    nc.vector.tensor_tensor(
                        out=lsm16[:].rearrange("p (b a) -> p b a", b=B).rearrange("p b a -> p a b"),
                        in0=ev,
                        in1=ssum[:].unsqueeze(1).to_broadcast([P, A, B]),
                        op=ALU.mult)
                else:
                    ev = elog.rearrange("p (b a) -> p b a", b=B)
                    nc.vector.tensor_reduce(out=ssum[:], in_=ev,
                                            axis=mybir.AxisListType.X, op=ALU.add)
                    nc.vector.reciprocal(out=ssum[:], in_=ssum[:])
                    nc.vector.tensor_tensor(
                        out=lsm16[:].rearrange("p (b a) -> p b a", b=B),
                        in0=ev,
                        in1=ssum[:].unsqueeze(2).to_broadcast([P, B, A]),
                        op=ALU.mult)

            def emit_preds(v16b4, lsm16, praw):
                """praw (c,b) f32 = sum_a lsm*V ; DVE c[0:CSPL], Pool rest."""
                pr4 = SCR["prod16"].rearrange("p (c b a) -> p c b a", c=C, b=B)
                t14 = SCR["tree1"].rearrange("p (c b a) -> p c b a", c=C, b=B)
                t24 = SCR["tree2"].rearrange("p (c b a) -> p c b a", c=C, b=B)
                t34 = SCR["tree3"].rearrange("p (c b a) -> p c b a", c=C, b=B)
                t44 = SCR["tree4"].rearrange("p (c b a) -> p c b a", c=C, b=B)
                prv = praw.rearrange("p (c b) -> p c b", c=C)
                for eng, cs in ((nc.vector, slice(0, CSPL)), (nc.gpsimd, slice(CSPL, C))):
                    ncs = cs.stop - cs.start
                    lbb = lsm16[:].rearrange("p (b a) -> p b a", b=B) \
                        .unsqueeze(1).to_broadcast([P, ncs, B, A])
                    eng.tensor_tensor(out=pr4[:, cs, :, :], in0=v16b4[:, cs, :, :],
                                      in1=lbb, op=ALU.mult)
                    eng.tensor_tensor(out=t14[:, cs, :, :], in0=pr4[:, cs, :, 0:16],
                                      in1=pr4[:, cs, :, 16:32], op=ALU.add)
                    eng.tensor_tensor(out=t24[:, cs, :, :], in0=t14[:, cs, :, 0:8],
                                      in1=t14[:, cs, :, 8:16], op=ALU.add)
                    eng.tensor_tensor(out=t34[:, cs, :, :], in0=t24[:, cs, :, 0:4],
                                      in1=t24[:, cs, :, 4:8], op=ALU.add)
                    eng.tensor_tensor(out=t44[:, cs, :, :], in0=t34[:, cs, :, 0:2],
                                      in1=t34[:, cs, :, 2:4], op=ALU.add)
                    eng.tensor_tensor(out=prv[:, cs, :], in0=t44[:, cs, :, 0],
                                      in1=t44[:, cs, :, 1], op=ALU.add)

            def emit_delta2(v16b4, pcur2, lsm16, logits2):
                """logits2 (b,a) f32 = sum_c V*P2 + lsm2 ; split by b."""
                t44 = SCR["tree4"].rearrange("p (b a) -> p b a", b=B)
                t4 = SCR["tree4"]
                pr4 = SCR["prod16"].rearrange("p (c b a) -> p c b a", c=C, b=B)
                t14 = SCR["tree1"].rearrange("p (c b a) -> p c b a", c=8, b=B)
                t24 = SCR["tree2"].rearrange("p (c b a) -> p c b a", c=4, b=B)
                t34 = SCR["tree3"].rearrange("p (c b a) -> p c b a", c=2, b=B)
                p2v = pcur2.rearrange("p (c b) -> p c b", c=C)
                for eng, bs in ((nc.vector, slice(0, BSPL)), (nc.gpsimd, slice(BSPL, B))):
                    nbs = bs.stop - bs.start
                    p2b = p2v[:, :, bs].unsqueeze(3).to_broadcast([P, C, nbs, A])
                    eng.tensor_tensor(out=pr4[:, :, bs, :], in0=v16b4[:, :, bs, :],
                                      in1=p2b, op=ALU.mult)
                    eng.tensor_tensor(out=t14[:, :, bs, :], in0=pr4[:, 0:8, bs, :],
                                      in1=pr4[:, 8:16, bs, :], op=ALU.add)
                    eng.tensor_tensor(out=t24[:, :, bs, :], in0=t14[:, 0:4, bs, :],
                                      in1=t14[:, 4:8, bs, :], op=ALU.add)
                    eng.tensor_tensor(out=t34[:, :, bs, :], in0=t24[:, 0:2, bs, :],
                                      in1=t24[:, 2:4, bs, :], op=ALU.add)
                    eng.tensor_tensor(out=t44[:, bs, :], in0=t34[:, 0, bs, :],
                                      in1=t34[:, 1, bs, :], op=ALU.add)
                nc.vector.tensor_tensor(out=logits2[:], in0=t4, in1=lsm16[:], op=ALU.add)

            # ================== main schedule ==================
            p1raw0, v16b40, v32gs0 = emit_votes(0)
            pcur1_0 = squash(p1raw0[:], "pcur1", "sq")
            logits = rt.tile([P, A * B], F32, tag="logits")
            if num_routes >= 2:
                emit_delta1(v32gs0, pcur1_0, logits, range(NG))
                nc.vector.tensor_scalar_add(out=logits[:], in0=logits[:], scalar1=1.0 / A)
            state = {"v16b4": v16b40, "pcur1": pcur1_0}

            for k in range(NCH):
                v16b4 = state["v16b4"]
                pcur1 = state["pcur1"]
                nxt = k + 1 < NCH
                p1raw_n = pcur1_n = v16b4_n = v32gs_n = None
                if nxt:
                    p1raw_n, v16b4_n, v32gs_n = emit_votes(k + 1)
                sq1_done = False

                def inject(groups, first=False):
                    nonlocal sq1_done, pcur1_n
                    if not nxt or num_routes < 2:
                        return
                    if not sq1_done:
                        pcur1_n = squash(p1raw_n[:], "pcur1", "sq")
                        sq1_done = True
                    emit_delta1(v32gs_n, pcur1_n, logits, groups)

                if num_routes == 1:
                    pcur_fin = pcur1
                    inject(range(NG))
                else:
                    layout = "ab"
                    lsm16 = rt.tile([P, B * A], F16, tag="lsm16")
                    pcur_fin = None
                    for it in range(2, num_routes + 1):
                        softmax(logits, layout, lsm16)
                        praw = SCR["praw"]
                        emit_preds(v16b4, lsm16, praw)
                        last = (it == num_routes)
                        if last:
                            inject(range(0, 3))
                        pcur = squash(praw, "pcurI", "sq")
                        if not last:
                            emit_delta2(v16b4, pcur, lsm16, logits)
                            layout = "ba"
                        else:
                            pcur_fin = pcur
                        if last:
                            inject(range(3, 6))

                nc.sync.dma_start(out=out_d.ap()[k * NCHUNK:(k + 1) * NCHUNK, :],
                                  in_=pcur_fin)

                if nxt:
                    inject(range(6, NG))
                    if num_routes >= 2:
                        nc.vector.tensor_scalar_add(out=logits[:], in0=logits[:],
                                                    scalar1=1.0 / A)
                    state = {"v16b4": v16b4_n, "pcur1": pcur1_n}

    nc.compile()
    return nc


def _prep(x, weights):
    xp = np.zeros((A, H + 2, W + 2), dtype=np.float32)
    xp[:, 1:-1, 1:-1] = x
    wvf = np.ascontiguousarray(weights.reshape(9, A, C * B))  # (c,b): idx c*B+b
    wv_h = wvf.astype(np.float16)
    wv_l = (wvf - wv_h.astype(np.float32)).astype(np.float16)
    # wp1 packed [96, 2, 3, CB]: row = (pq_local*32 + a) for q-block
    wp1 = np.empty((96, 2, 3, CB), dtype=np.float16)
    for q in range(3):
        wp1[:, 0, q, :] = wv_h[3 * q:3 * q + 3].reshape(96, CB)
        wp1[:, 1, q, :] = wv_l[3 * q:3 * q + 3].reshape(96, CB)
    in_maps = []
    for core in range(NCORES):
        r0 = core * ROWS
        pat = np.empty((9, A, ROWS, W), dtype=np.float32)
        for dp in range(3):
            for dq in range(3):
                pat[dp * 3 + dq] = xp[:, r0 + dp:r0 + dp + ROWS, dq:dq + W]
        patf = np.ascontiguousarray(pat.reshape(9, A, NPOS))
        pat_h = patf.astype(np.float16)
        pat_l = (patf - pat_h.astype(np.float32)).astype(np.float16)
        pat2 = np.stack([pat_h, pat_l], axis=2)  # [9, A, 2, NPOS]
        pp1 = np.empty((96, 2, 3, NPOS), dtype=np.float16)
        for q in range(3):
            pp1[:, 0, q, :] = pat_h[3 * q:3 * q + 3].reshape(96, NPOS)
            pp1[:, 1, q, :] = pat_l[3 * q:3 * q + 3].reshape(96, NPOS)
        in_maps.append({"pat2": np.ascontiguousarray(pat2), "wvh": wv_h, "wvl": wv_l,
                        "pp1": pp1, "wp1": wp1})
    return in_maps


def kernel(x=None, weights=None, num_routes=3, **kw):
    x = np.asarray(x, dtype=np.float32)
    weights = np.asarray(weights, dtype=np.float32)
    nr = int(num_routes)
    if nr not in _CACHE:
        _CACHE[nr] = _build_nc(nr)
    nc = _CACHE[nr]
    in_maps = _prep(x, weights)
    res = bass_utils.run_bass_kernel_spmd(nc, in_maps, core_ids=list(range(NCORES)))
    out = np.empty((B, C, H, W), dtype=np.float32)
    for core in range(NCORES):
        o = np.asarray(res.results[core]["out"]).reshape(ROWS, W, C, B)
        out[:, :, core * ROWS:(core + 1) * ROWS, :] = o.transpose(3, 2, 0, 1)
    return out


def profile_once(inputs):
    from concourse import bass_interp
    nr = int(inputs.get("num_routes", 3))
    if nr not in _CACHE:
        _CACHE[nr] = _build_nc(nr)
    sim = bass_interp.CoreSim(_CACHE[nr], no_exec=True, ignore_data_errors=True,
                              publish_trace=False)
    sim.simulate()
    return int(sim.time)


# revision 4
# speedup vs baseline: 2.2001x; 1.0330x over previous
import os, sys
import numpy as np

sys.path.insert(0, "/opt/trn_rl_repo")

from concourse import bass, bacc, bass_utils
from concourse import mybir
from concourse.tile import TileContext

F32 = mybir.dt.float32
F16 = mybir.dt.float16
ALU = mybir.AluOpType
ACTF = mybir.ActivationFunctionType

A = 32          # in_maps
B = 32          # out_maps
C = 16          # atoms
H = 64
W = 64
NCORES = 8
ROWS = H // NCORES
NPOS = ROWS * W             # 512 positions per core
NCHUNK = 128
NCH = NPOS // NCHUNK        # 4 chunks
CB = C * B                  # 512, (c,b)-order: idx = c*B + b
EPS = 1e-4
AG = 4                      # a-group size for PSUM staging
NG = A // AG                # 8 groups
P = NCHUNK

# stage split (DVE : Pool)
BSPL = 16                   # delta2 b-split: DVE gets b[0:BSPL]
CSPL = 10                   # preds c-split: DVE gets c[0:CSPL]
GSPL = 5                    # delta1 group split: DVE gets groups [0:GSPL]

_CACHE = {}


def _build_nc(num_routes: int):
    nc = bacc.Bacc(None, target_bir_lowering=False)

    pat2_d = nc.declare_dram_parameter("pat2", [9, A, 2, NPOS], F16, isOutput=False)
    wvh_d = nc.declare_dram_parameter("wvh", [9, A, CB], F16, isOutput=False)
    wvl_d = nc.declare_dram_parameter("wvl", [9, A, CB], F16, isOutput=False)
    pp1_d = nc.declare_dram_parameter("pp1", [96, 2, 3, NPOS], F16, isOutput=False)
    wp1_d = nc.declare_dram_parameter("wp1", [96, 2, 3, CB], F16, isOutput=False)
    out_d = nc.declare_dram_parameter("out", [NPOS, CB], F32, isOutput=True)

    with TileContext(nc) as tc:
        with (
            tc.tile_pool(name="wp1", bufs=1) as wp1pool,
            tc.tile_pool(name="win", bufs=2) as winpool,
            tc.tile_pool(name="v16", bufs=2) as v16pool,
            tc.tile_pool(name="v32", bufs=2) as v32pool,
            tc.tile_pool(name="psv", bufs=2, space="PSUM") as psv,
            tc.tile_pool(name="rt", bufs=1) as rt,
        ):
            # ---- static p1 weights [96, (hl q n)] one DMA
            wp1t = wp1pool.tile([96, 2 * 3 * CB], F16, name="wp1t", tag="wp1t")
            nc.sync.dma_start(out=wp1t[:].rearrange("p (h q n) -> p h q n", h=2, q=3),
                              in_=wp1_d.ap())
            wp1v = wp1t[:].rearrange("p (h q n) -> p h q n", h=2, q=3)

            # ---- resident hi weights [9, A*CB] f16 (slices loaded lazily)
            wvh = wp1pool.tile([9, A * CB], F16, name="wvh", tag="wvh")
            wvh3 = wvh[:].rearrange("q (a n) -> q a n", a=A)
            wvh_loaded = [False] * 4

            def ensure_wvh(qw):
                if not wvh_loaded[qw]:
                    asl = slice(qw * 8, (qw + 1) * 8)
                    nc.sync.dma_start(
                        out=wvh3[:, asl, :],
                        in_=wvh_d.ap()[:, asl, :])
                    wvh_loaded[qw] = True

            # ---- shared routing scratch (aliased: delta1 f32 views live in
            # scrA/scrB/tr2/tr3 whose f16 users are temporally disjoint)
            scrA = rt.tile([P, 16384], F16, tag="scrA")   # prod16 | pd1v/pd1g
            scrB = rt.tile([P, 8192], F16, tag="scrB")    # tree1  | t1d1v/g | praw/t3d1
            tr2 = rt.tile([P, 4096], F16, tag="tr2")      # tree2  | t2d1v/g | elog
            tr3 = rt.tile([P, 2048], F16, tag="tr3")      # tree3
            tr4 = rt.tile([P, 1024], F16, tag="tr4")      # tree4
            pcur1t = rt.tile([P, CB], F32, tag="pcur1t")
            SCR = {
                "prod16": scrA[:],
                "tree1": scrB[:],
                "tree2": tr2[:],
                "tree3": tr3[:],
                "tree4": tr4[:],
                "sqp2": tr4[:, 0:1024].bitcast(F32),
                "pd1v": scrA[:, 0:4096].bitcast(F32),
                "pd1g": scrA[:, 4096:8192].bitcast(F32),
                "t1d1v": scrB[:, 0:2048].bitcast(F32),
                "t1d1g": scrB[:, 2048:4096].bitcast(F32),
                "t2d1v": tr2[:, 0:1024].bitcast(F32),
                "t2d1g": tr2[:, 1024:2048].bitcast(F32),
                "t3d1v": scrB[:, 5120:5632].bitcast(F32),
                "t3d1g": scrB[:, 5632:6144].bitcast(F32),
                "elog": tr2[:, 2048:4096].bitcast(F32),
                "praw": scrB[:, 4096:5120].bitcast(F32),
                "pcur1": pcur1t[:],
            }

            def emit_votes(k):
                """DMA + PE voting + Act drains for chunk k."""
                ksl = slice(k * NCHUNK, (k + 1) * NCHUNK)
                pp1t = winpool.tile([96, 2 * 3 * NCHUNK], F16, name="pp1t", tag="pp1t", bufs=1)
                nc.sync.dma_start(
                    out=pp1t[:].rearrange("p (h q n) -> p h q n", h=2, q=3),
                    in_=pp1_d.ap()[:, :, :, ksl])
                pp1v = pp1t[:].rearrange("p (h q n) -> p h q n", h=2, q=3)
                # p1 matmuls into a shared psv-slot (uses first bank only)
                p1t = psv.tile([P, AG * CB], F32, tag="vps")
                p1ps = p1t[:, :CB]
                for q in range(3):
                    nc.tensor.matmul(out=p1ps, lhsT=pp1v[:, 0, q, :], rhs=wp1v[:, 0, q, :],
                                     start=(q == 0), stop=False)
                    nc.tensor.matmul(out=p1ps, lhsT=pp1v[:, 0, q, :], rhs=wp1v[:, 1, q, :],
                                     start=False, stop=False)
                    nc.tensor.matmul(out=p1ps, lhsT=pp1v[:, 1, q, :], rhs=wp1v[:, 0, q, :],
                                     start=False, stop=(q == 2))
                p1raw = rt.tile([P, CB], F32, tag="p1raw")
                nc.scalar.mul(out=p1raw[:], in_=p1ps, mul=1.0 / A)

                v16b = v16pool.tile([P, CB * A], F16, tag="v16b")   # (c,b,a)
                v16b4 = v16b[:].rearrange("p (c b a) -> p c b a", c=C, b=B)
                v32gs = []
                for g in range(NG):
                    w0 = g * AG
                    pkt = winpool.tile([9, AG * 2 * NCHUNK], F16, name="pk", tag="pk")
                    nc.sync.dma_start(
                        out=pkt[:].rearrange("q (a h n) -> q a h n", a=AG, h=2),
                        in_=pat2_d.ap()[:, w0:w0 + AG, :, ksl])
                    pkv = pkt[:].rearrange("q (a h n) -> q a h n", a=AG, h=2)
                    if True:
                        wlt = winpool.tile([9, AG * CB], F16, name="wgl", tag="wgl")
                        nc.sync.dma_start(
                            out=wlt[:].rearrange("q (a n) -> q a n", a=AG),
                            in_=wvl_d.ap()[:, w0:w0 + AG, :])
                        wl3 = wlt[:].rearrange("q (a n) -> q a n", a=AG)
                    ensure_wvh(g // 2)
                    vps = psv.tile([P, AG * CB], F32, tag="vps")
                    for ai in range(AG):
                        a_glob = g * AG + ai
                        aw = ai
                        osl = vps[:, ai * CB:(ai + 1) * CB]
                        nc.tensor.matmul(out=osl, lhsT=pkv[:, ai, 0, :], rhs=wvh3[:, a_glob, :],
                                         start=True, stop=False)
                        nc.tensor.matmul(out=osl, lhsT=pkv[:, ai, 0, :], rhs=wl3[:, aw, :],
                                         start=False, stop=False)
                        nc.tensor.matmul(out=osl, lhsT=pkv[:, ai, 1, :], rhs=wvh3[:, a_glob, :],
                                         start=False, stop=True)
                    for ai in range(AG):
                        a_glob = g * AG + ai
                        nc.scalar.copy(
                            out=v16b4[:, :, :, a_glob],
                            in_=vps[:, ai * CB:(ai + 1) * CB].rearrange("p (c b) -> p c b", c=C))
                    v32g = v32pool.tile([P, AG * CB], F32, tag="v32g")
                    nc.scalar.copy(out=v32g[:], in_=vps[:])
                    v32gs.append(v32g)
                return p1raw, v16b4, v32gs

            def squash(praw_cb, pcur_tag, sq_tag):
                p2 = SCR["sqp2"]
                nc.scalar.activation(out=p2, in_=praw_cb, func=ACTF.Square)
                s = rt.tile([P, B], F32, tag=f"{sq_tag}s")
                nc.vector.tensor_reduce(
                    out=s[:], in_=p2.rearrange("p (c b) -> p b c", c=C),
                    axis=mybir.AxisListType.X, op=ALU.add)
                nc.vector.tensor_scalar_add(out=s[:], in0=s[:], scalar1=EPS)  # s = sq
                nrm = rt.tile([P, B], F32, tag=f"{sq_tag}n")
                nc.scalar.activation(out=nrm[:], in_=s[:], func=ACTF.Sqrt)
                fac = rt.tile([P, B], F32, tag=f"{sq_tag}f")
                nc.vector.scalar_tensor_tensor(out=fac[:], in0=s[:], scalar=1.0,
                                               in1=nrm[:], op0=ALU.add, op1=ALU.mult)
                nc.vector.tensor_scalar_add(out=fac[:], in0=fac[:], scalar1=EPS)
                nc.vector.reciprocal(out=fac[:], in_=fac[:])
                nc.vector.tensor_tensor(out=fac[:], in0=s[:], in1=fac[:], op=ALU.mult)
                if pcur_tag == "pcur1":
                    pcur_ap = SCR["pcur1"]
                else:
                    pcur_t = rt.tile([P, CB], F32, tag=pcur_tag, name=pcur_tag)
                    pcur_ap = pcur_t[:]
                nc.vector.tensor_tensor(
                    out=pcur_ap.rearrange("p (c b) -> p c b", c=C),
                    in0=praw_cb.rearrange("p (c b) -> p c b", c=C),
                    in1=fac[:].unsqueeze(1).to_broadcast([P, C, B]),
                    op=ALU.mult)
                return pcur_ap

            def emit_delta1(v32gs, pcur1, logits1, groups):
                """logits1 (a,b) f32 = sum_c V*P1 (groups subset)."""
                l1v = logits1[:].rearrange("p (a b) -> p a b", a=A)
                p1b = pcur1.rearrange("p (c b) -> p c b", c=C) \
                    .unsqueeze(1).to_broadcast([P, AG, C, B])
                for g in groups:
                    eng = nc.vector if g < GSPL else nc.gpsimd
                    sfx = "v" if g < GSPL else "g"
                    v32g4 = v32gs[g][:].rearrange("p (a c b) -> p a c b", a=AG, c=C)
                    pd4 = SCR[f"pd1{sfx}"].rearrange("p (a c b) -> p a c b", a=AG, c=C)
                    eng.tensor_tensor(out=pd4, in0=v32g4, in1=p1b, op=ALU.mult)
                    t14 = SCR[f"t1d1{sfx}"].rearrange("p (a c b) -> p a c b", a=AG, c=8)
                    eng.tensor_tensor(out=t14, in0=pd4[:, :, 0:8, :], in1=pd4[:, :, 8:16, :], op=ALU.add)
                    t24 = SCR[f"t2d1{sfx}"].rearrange("p (a c b) -> p a c b", a=AG, c=4)
                    eng.tensor_tensor(out=t24, in0=t14[:, :, 0:4, :], in1=t14[:, :, 4:8, :], op=ALU.add)
                    t34 = SCR[f"t3d1{sfx}"].rearrange("p (a c b) -> p a c b", a=AG, c=2)
                    eng.tensor_tensor(out=t34, in0=t24[:, :, 0:2, :], in1=t24[:, :, 2:4, :], op=ALU.add)
                    eng.tensor_tensor(out=l1v[:, g * AG:(g + 1) * AG, :],
                                      in0=t34[:, :, 0, :], in1=t34[:, :, 1, :], op=ALU.add)

            def softmax(logits, layout, lsm16):
                elog = SCR["elog"]
                nc.scalar.activation(out=elog, in_=logits[:], func=ACTF.Exp)
                ssum = rt.tile([P, B], F32, tag="sqs")
                if layout == "ab":
                    ev = elog.rearrange("p (a b) -> p a b", a=A)
                    nc.vector.tensor_reduce(out=ssum[:], in_=ev.rearrange("p a b -> p b a"),
                                            axis=mybir.AxisListType.X, op=ALU.add)
                    nc.vector.reciprocal(out=ssum[:], in_=ssum[:])
                    nc.vector.tensor_tensor(
                        out=lsm16[:].rearrange("p (b a) -> p b a", b=B).rearrange("p b a -> p a b"),
                        in0=ev,
                        in1=ssum[:].unsqueeze(1).to_broadcast([P, A, B]),
                        op=ALU.mult)
                else:
                    ev = elog.rearrange("p (b a) -> p b a", b=B)
                    nc.vector.tensor_reduce(out=ssum[:], in_=ev,
                                            axis=mybir.AxisListType.X, op=ALU.add)
                    nc.vector.reciprocal(out=ssum[:], in_=ssum[:])
                    nc.vector.tensor_tensor(
                        out=lsm16[:].rearrange("p (b a) -> p b a", b=B),
                        in0=ev,
                        in1=ssum[:].unsqueeze(2).to_broadcast([P, B, A]),
                        op=ALU.mult)

            def emit_preds(v16b4, lsm16, praw):
                """praw (c,b) f32 = sum_a lsm*V ; DVE c[0:CSPL], Pool rest."""
                pr4 = SCR["prod16"].rearrange("p (c b a) -> p c b a", c=C, b=B)
                t14 = SCR["tree1"].rearrange("p (c b a) -> p c b a", c=C, b=B)
                t24 = SCR["tree2"].rearrange("p (c b a) -> p c b a", c=C, b=B)
                t34 = SCR["tree3"].rearrange("p (c b a) -> p c b a", c=C, b=B)
                t44 = SCR["tree4"].rearrange("p (c b a) -> p c b a", c=C, b=B)
                prv = praw.rearrange("p (c b) -> p c b", c=C)
                for eng, cs in ((nc.vector, slice(0, CSPL)), (nc.gpsimd, slice(CSPL, C))):
                    ncs = cs.stop - cs.start
                    lbb = lsm16[:].rearrange("p (b a) -> p b a", b=B) \
                        .unsqueeze(1).to_broadcast([P, ncs, B, A])
                    eng.tensor_tensor(out=pr4[:, cs, :, :], in0=v16b4[:, cs, :, :],
                                      in1=lbb, op=ALU.mult)
                    eng.tensor_tensor(out=t14[:, cs, :, :], in0=pr4[:, cs, :, 0:16],
                                      in1=pr4[:, cs, :, 16:32], op=ALU.add)
                    eng.tensor_tensor(out=t24[:, cs, :, :], in0=t14[:, cs, :, 0:8],
                                      in1=t14[:, cs, :, 8:16], op=ALU.add)
                    eng.tensor_tensor(out=t34[:, cs, :, :], in0=t24[:, cs, :, 0:4],
                                      in1=t24[:, cs, :, 4:8], op=ALU.add)
                    eng.tensor_tensor(out=t44[:, cs, :, :], in0=t34[:, cs, :, 0:2],
                                      in1=t34[:, cs, :, 2:4], op=ALU.add)
                    eng.tensor_tensor(out=prv[:, cs, :], in0=t44[:, cs, :, 0],
                                      in1=t44[:, cs, :, 1], op=ALU.add)

            def emit_delta2(v16b4, pcur2, lsm16, logits2):
                """logits2 (b,a) f32 = sum_c V*P2 + lsm2 ; split by b."""
                t44 = SCR["tree4"].rearrange("p (b a) -> p b a", b=B)
                t4 = SCR["tree4"]
                pr4 = SCR["prod16"].rearrange("p (c b a) -> p c b a", c=C, b=B)
                t14 = SCR["tree1"].rearrange("p (c b a) -> p c b a", c=8, b=B)
                t24 = SCR["tree2"].rearrange("p (c b a) -> p c b a", c=4, b=B)
                t34 = SCR["tree3"].rearrange("p (c b a) -> p c b a", c=2, b=B)
                p2v = pcur2.rearrange("p (c b) -> p c b", c=C)
                for eng, bs in ((nc.vector, slice(0, BSPL)), (nc.gpsimd, slice(BSPL, B))):
                    nbs = bs.stop - bs.start
                    p2b = p2v[:, :, bs].unsqueeze(3).to_broadcast([P, C, nbs, A])
                    eng.tensor_tensor(out=pr4[:, :, bs, :], in0=v16b4[:, :, bs, :],
                                      in1=p2b, op=ALU.mult)
                    eng.tensor_tensor(out=t14[:, :, bs, :], in0=pr4[:, 0:8, bs, :],
                                      in1=pr4[:, 8:16, bs, :], op=ALU.add)
                    eng.tensor_tensor(out=t24[:, :, bs, :], in0=t14[:, 0:4, bs, :],
                                      in1=t14[:, 4:8, bs, :], op=ALU.add)
                    eng.tensor_tensor(out=t34[:, :, bs, :], in0=t24[:, 0:2, bs, :],
                                      in1=t24[:, 2:4, bs, :], op=ALU.add)
                    eng.tensor_tensor(out=t44[:, bs, :], in0=t34[:, 0, bs, :],
                                      in1=t34[:, 1, bs, :], op=ALU.add)
                nc.vector.tensor_tensor(out=logits2[:], in0=t4, in1=lsm16[:], op=ALU.add)

            # ================== main schedule ==================
            p1raw0, v16b40, v32gs0 = emit_votes(0)
            pcur1_0 = squash(p1raw0[:], "pcur1", "sq")
            logits = rt.tile([P, A * B], F32, tag="logits")
            if num_routes >= 2:
                emit_delta1(v32gs0, pcur1_0, logits, range(NG))
                nc.vector.tensor_scalar_add(out=logits[:], in0=logits[:], scalar1=1.0 / A)
            state = {"v16b4": v16b40, "pcur1": pcur1_0}

            for k in range(NCH):
                v16b4 = state["v16b4"]
                pcur1 = state["pcur1"]
                nxt = k + 1 < NCH
                p1raw_n = pcur1_n = v16b4_n = v32gs_n = None
                if nxt:
                    p1raw_n, v16b4_n, v32gs_n = emit_votes(k + 1)
                sq1_done = False

                def inject(groups, first=False):
                    nonlocal sq1_done, pcur1_n
                    if not nxt or num_routes < 2:
                        return
                    if not sq1_done:
                        pcur1_n = squash(p1raw_n[:], "pcur1", "sq")
                        sq1_done = True
                    emit_delta1(v32gs_n, pcur1_n, logits, groups)

                if num_routes == 1:
                    pcur_fin = pcur1
                    inject(range(NG))
                else:
                    layout = "ab"
                    lsm16 = rt.tile([P, B * A], F16, tag="lsm16")
                    pcur_fin = None
                    for it in range(2, num_routes + 1):
                        softmax(logits, layout, lsm16)
                        praw = SCR["praw"]
                        emit_preds(v16b4, lsm16, praw)
                        last = (it == num_routes)
                        if last:
                            inject(range(0, 3))
                        pcur = squash(praw, "pcurI", "sq")
                        if not last:
                            emit_delta2(v16b4, pcur, lsm16, logits)
                            layout = "ba"
                        else:
                            pcur_fin = pcur
                        if last:
                            inject(range(3, 6))

                nc.sync.dma_start(out=out_d.ap()[k * NCHUNK:(k + 1) * NCHUNK, :],
                                  in_=pcur_fin)

                if nxt:
                    inject(range(6, NG))
                    if num_routes >= 2:
                        nc.vector.tensor_scalar_add(out=logits[:], in0=logits[:],
                                                    scalar1=1.0 / A)
                    state = {"v16b4": v16b4_n, "pcur1": pcur1_n}

    nc.compile()
    return nc


def _prep(x, weights):
    xp = np.zeros((A, H + 2, W + 2), dtype=np.float32)
    xp[:, 1:-1, 1:-1] = x
    wvf = np.ascontiguousarray(weights.reshape(9, A, C * B))  # (c,b): idx c*B+b
    wv_h = wvf.astype(np.float16)
    wv_l = (wvf - wv_h.astype(np.float32)).astype(np.float16)
    # wp1 packed [96, 2, 3, CB]: row = (pq_local*32 + a) for q-block
    wp1 = np.empty((96, 2, 3, CB), dtype=np.float16)
    for q in range(3):
        wp1[:, 0, q, :] = wv_h[3 * q:3 * q + 3].reshape(96, CB)
        wp1[:, 1, q, :] = wv_l[3 * q:3 * q + 3].reshape(96, CB)
    in_maps = []
    for core in range(NCORES):
        r0 = core * ROWS
        pat = np.empty((9, A, ROWS, W), dtype=np.float32)
        for dp in range(3):
            for dq in range(3):
                pat[dp * 3 + dq] = xp[:, r0 + dp:r0 + dp + ROWS, dq:dq + W]
        patf = np.ascontiguousarray(pat.reshape(9, A, NPOS))
        pat_h = patf.astype(np.float16)
        pat_l = (patf - pat_h.astype(np.float32)).astype(np.float16)
        pat2 = np.stack([pat_h, pat_l], axis=2)  # [9, A, 2, NPOS]
        pp1 = np.empty((96, 2, 3, NPOS), dtype=np.float16)
        for q in range(3):
            pp1[:, 0, q, :] = pat_h[3 * q:3 * q + 3].reshape(96, NPOS)
            pp1[:, 1, q, :] = pat_l[3 * q:3 * q + 3].reshape(96, NPOS)
        in_maps.append({"pat2": np.ascontiguousarray(pat2), "wvh": wv_h, "wvl": wv_l,
                        "pp1": pp1, "wp1": wp1})
    return in_maps


def kernel(x=None, weights=None, num_routes=3, **kw):
    x = np.asarray(x, dtype=np.float32)
    weights = np.asarray(weights, dtype=np.float32)
    nr = int(num_routes)
    if nr not in _CACHE:
        _CACHE[nr] = _build_nc(nr)
    nc = _CACHE[nr]
    in_maps = _prep(x, weights)
    res = bass_utils.run_bass_kernel_spmd(nc, in_maps, core_ids=list(range(NCORES)))
    out = np.empty((B, C, H, W), dtype=np.float32)
    for core in range(NCORES):
        o = np.asarray(res.results[core]["out"]).reshape(ROWS, W, C, B)
        out[:, :, core * ROWS:(core + 1) * ROWS, :] = o.transpose(3, 2, 0, 1)
    return out


def profile_once(inputs):
    from concourse import bass_interp
    nr = int(inputs.get("num_routes", 3))
    if nr not in _CACHE:
        _CACHE[nr] = _build_nc(nr)
    sim = bass_interp.CoreSim(_CACHE[nr], no_exec=True, ignore_data_errors=True,
                              publish_trace=False)
    sim.simulate()
    return int(sim.time)


# revision 5
# speedup vs baseline: 2.2043x; 1.0019x over previous
import os, sys
import numpy as np

sys.path.insert(0, "/opt/trn_rl_repo")

from concourse import bass, bacc, bass_utils
from concourse import mybir
from concourse.tile import TileContext

F32 = mybir.dt.float32
F16 = mybir.dt.float16
ALU = mybir.AluOpType
ACTF = mybir.ActivationFunctionType

A = 32          # in_maps
B = 32          # out_maps
C = 16          # atoms
H = 64
W = 64
NCORES = 8
ROWS = H // NCORES
NPOS = ROWS * W             # 512 positions per core
NCHUNK = 128
NCH = NPOS // NCHUNK        # 4 chunks
CB = C * B                  # 512, (c,b)-order: idx = c*B + b
EPS = 1e-4
AG = 4                      # a-group size for PSUM staging
NG = A // AG                # 8 groups
P = NCHUNK

# stage split (DVE : Pool)
BSPL = 16                   # delta2 b-split: DVE gets b[0:BSPL]
CSPL = 10                   # preds c-split: DVE gets c[0:CSPL]
GSPL = 5                    # delta1 group split: DVE gets groups [0:GSPL]

_CACHE = {}


def _build_nc(num_routes: int):
    nc = bacc.Bacc(None, target_bir_lowering=False)

    pat2_d = nc.declare_dram_parameter("pat2", [9, A, 2, NPOS], F16, isOutput=False)
    wvh_d = nc.declare_dram_parameter("wvh", [9, A, CB], F16, isOutput=False)
    wvl_d = nc.declare_dram_parameter("wvl", [9, A, CB], F16, isOutput=False)
    pp1_d = nc.declare_dram_parameter("pp1", [96, 2, 3, NPOS], F16, isOutput=False)
    wp1_d = nc.declare_dram_parameter("wp1", [96, 2, 3, CB], F16, isOutput=False)
    out_d = nc.declare_dram_parameter("out", [NPOS, CB], F32, isOutput=True)

    with TileContext(nc) as tc:
        with (
            tc.tile_pool(name="wp1", bufs=1) as wp1pool,
            tc.tile_pool(name="win", bufs=2) as winpool,
            tc.tile_pool(name="v16", bufs=2) as v16pool,
            tc.tile_pool(name="v32", bufs=2) as v32pool,
            tc.tile_pool(name="psv", bufs=2, space="PSUM") as psv,
            tc.tile_pool(name="rt", bufs=1) as rt,
        ):
            # ---- static p1 weights [96, (hl q n)] one DMA
            wp1t = wp1pool.tile([96, 2 * 3 * CB], F16, name="wp1t", tag="wp1t")
            nc.sync.dma_start(out=wp1t[:].rearrange("p (h q n) -> p h q n", h=2, q=3),
                              in_=wp1_d.ap())
            wp1v = wp1t[:].rearrange("p (h q n) -> p h q n", h=2, q=3)

            # ---- resident hi weights [9, A*CB] f16 (slices loaded lazily)
            wvh = wp1pool.tile([9, A * CB], F16, name="wvh", tag="wvh")
            wvh3 = wvh[:].rearrange("q (a n) -> q a n", a=A)
            wvh_loaded = [False] * 4

            def ensure_wvh(qw):
                if not wvh_loaded[qw]:
                    asl = slice(qw * 8, (qw + 1) * 8)
                    nc.sync.dma_start(
                        out=wvh3[:, asl, :],
                        in_=wvh_d.ap()[:, asl, :])
                    wvh_loaded[qw] = True

            # ---- shared routing scratch (aliased: delta1 f32 views live in
            # scrA/scrB/tr2/tr3 whose f16 users are temporally disjoint)
            scrA = rt.tile([P, 16384], F16, tag="scrA")   # prod16 | pd1v/pd1g
            scrB = rt.tile([P, 8192], F16, tag="scrB")    # tree1  | t1d1v/g | praw/t3d1
            tr2 = rt.tile([P, 4096], F16, tag="tr2")      # tree2  | t2d1v/g | elog
            tr3 = rt.tile([P, 2048], F16, tag="tr3")      # tree3
            tr4 = rt.tile([P, 1024], F16, tag="tr4")      # tree4
            pcur1t = rt.tile([P, CB], F32, tag="pcur1t")
            SCR = {
                "prod16": scrA[:],
                "tree1": scrB[:],
                "tree2": tr2[:],
                "tree3": tr3[:],
                "tree4": tr4[:],
                "sqp2": tr4[:, 0:1024].bitcast(F32),
                "pd1v": scrA[:, 0:4096].bitcast(F32),
                "pd1g": scrA[:, 4096:8192].bitcast(F32),
                "t1d1v": scrB[:, 0:2048].bitcast(F32),
                "t1d1g": scrB[:, 2048:4096].bitcast(F32),
                "t2d1v": tr2[:, 0:1024].bitcast(F32),
                "t2d1g": tr2[:, 1024:2048].bitcast(F32),
                "t3d1v": scrB[:, 5120:5632].bitcast(F32),
                "t3d1g": scrB[:, 5632:6144].bitcast(F32),
                "elog": tr2[:, 2048:4096].bitcast(F32),
                "praw": scrB[:, 4096:5120].bitcast(F32),
                "pcur1": pcur1t[:],
            }

            def emit_votes(k):
                """DMA + PE voting + Act drains for chunk k."""
                ksl = slice(k * NCHUNK, (k + 1) * NCHUNK)
                pp1t = winpool.tile([96, 2 * 3 * NCHUNK], F16, name="pp1t", tag="pp1t", bufs=1)
                nc.sync.dma_start(
                    out=pp1t[:].rearrange("p (h q n) -> p h q n", h=2, q=3),
                    in_=pp1_d.ap()[:, :, :, ksl])
                pp1v = pp1t[:].rearrange("p (h q n) -> p h q n", h=2, q=3)
                # p1 matmuls into a shared psv-slot (uses first bank only)
                p1t = psv.tile([P, AG * CB], F32, tag="vps")
                p1ps = p1t[:, :CB]
                for q in range(3):
                    nc.tensor.matmul(out=p1ps, lhsT=pp1v[:, 0, q, :], rhs=wp1v[:, 0, q, :],
                                     start=(q == 0), stop=False)
                    nc.tensor.matmul(out=p1ps, lhsT=pp1v[:, 0, q, :], rhs=wp1v[:, 1, q, :],
                                     start=False, stop=False)
                    nc.tensor.matmul(out=p1ps, lhsT=pp1v[:, 1, q, :], rhs=wp1v[:, 0, q, :],
                                     start=False, stop=(q == 2))
                p1raw = rt.tile([P, CB], F32, tag="p1raw")
                nc.scalar.mul(out=p1raw[:], in_=p1ps, mul=1.0 / A)

                v16b = v16pool.tile([P, CB * A], F16, tag="v16b")   # (c,b,a)
                v16b4 = v16b[:].rearrange("p (c b a) -> p c b a", c=C, b=B)
                v32gs = []
                for g in range(NG):
                    w0 = g * AG
                    pkt = winpool.tile([9, AG * 2 * NCHUNK], F16, name="pk", tag="pk")
                    nc.sync.dma_start(
                        out=pkt[:].rearrange("q (a h n) -> q a h n", a=AG, h=2),
                        in_=pat2_d.ap()[:, w0:w0 + AG, :, ksl])
                    pkv = pkt[:].rearrange("q (a h n) -> q a h n", a=AG, h=2)
                    if True:
                        wlt = winpool.tile([9, AG * CB], F16, name="wgl", tag="wgl")
                        nc.sync.dma_start(
                            out=wlt[:].rearrange("q (a n) -> q a n", a=AG),
                            in_=wvl_d.ap()[:, w0:w0 + AG, :])
                        wl3 = wlt[:].rearrange("q (a n) -> q a n", a=AG)
                    ensure_wvh(g // 2)
                    vps = psv.tile([P, AG * CB], F32, tag="vps")
                    for ai in range(AG):
                        a_glob = g * AG + ai
                        aw = ai
                        osl = vps[:, ai * CB:(ai + 1) * CB]
                        nc.tensor.matmul(out=osl, lhsT=pkv[:, ai, 0, :], rhs=wvh3[:, a_glob, :],
                                         start=True, stop=False)
                        nc.tensor.matmul(out=osl, lhsT=pkv[:, ai, 0, :], rhs=wl3[:, aw, :],
                                         start=False, stop=False)
                        nc.tensor.matmul(out=osl, lhsT=pkv[:, ai, 1, :], rhs=wvh3[:, a_glob, :],
                                         start=False, stop=True)
                    for ai in range(AG):
                        a_glob = g * AG + ai
                        nc.scalar.copy(
                            out=v16b4[:, :, :, a_glob],
                            in_=vps[:, ai * CB:(ai + 1) * CB].rearrange("p (c b) -> p c b", c=C))
                    v32g = v32pool.tile([P, AG * CB], F32, tag="v32g")
                    nc.scalar.copy(out=v32g[:], in_=vps[:])
                    v32gs.append(v32g)
                return p1raw, v16b4, v32gs

            def squash(praw_cb, pcur_tag, sq_tag):
                p2 = SCR["sqp2"]
                nc.scalar.activation(out=p2, in_=praw_cb, func=ACTF.Square)
                s = rt.tile([P, B], F32, tag=f"{sq_tag}s")
                nc.vector.tensor_reduce(
                    out=s[:], in_=p2.rearrange("p (c b) -> p b c", c=C),
                    axis=mybir.AxisListType.X, op=ALU.add)
                nc.vector.tensor_scalar_add(out=s[:], in0=s[:], scalar1=EPS)  # s = sq
                nrm = rt.tile([P, B], F32, tag=f"{sq_tag}n")
                nc.scalar.activation(out=nrm[:], in_=s[:], func=ACTF.Sqrt)
                fac = rt.tile([P, B], F32, tag=f"{sq_tag}f")
                nc.vector.scalar_tensor_tensor(out=fac[:], in0=s[:], scalar=1.0,
                                               in1=nrm[:], op0=ALU.add, op1=ALU.mult)
                nc.vector.tensor_scalar_add(out=fac[:], in0=fac[:], scalar1=EPS)
                nc.vector.reciprocal(out=fac[:], in_=fac[:])
                nc.vector.tensor_tensor(out=fac[:], in0=s[:], in1=fac[:], op=ALU.mult)
                if pcur_tag == "pcur1":
                    pcur_ap = SCR["pcur1"]
                else:
                    pcur_t = rt.tile([P, CB], F32, tag=pcur_tag, name=pcur_tag)
                    pcur_ap = pcur_t[:]
                nc.vector.tensor_tensor(
                    out=pcur_ap.rearrange("p (c b) -> p c b", c=C),
                    in0=praw_cb.rearrange("p (c b) -> p c b", c=C),
                    in1=fac[:].unsqueeze(1).to_broadcast([P, C, B]),
                    op=ALU.mult)
                return pcur_ap

            def emit_delta1(v32gs, pcur1, logits1, groups):
                """logits1 (a,b) f32 = sum_c V*P1 (groups subset)."""
                l1v = logits1[:].rearrange("p (a b) -> p a b", a=A)
                p1b = pcur1.rearrange("p (c b) -> p c b", c=C) \
                    .unsqueeze(1).to_broadcast([P, AG, C, B])
                for g in groups:
                    eng = nc.vector if g < GSPL else nc.gpsimd
                    sfx = "v" if g < GSPL else "g"
                    v32g4 = v32gs[g][:].rearrange("p (a c b) -> p a c b", a=AG, c=C)
                    pd4 = SCR[f"pd1{sfx}"].rearrange("p (a c b) -> p a c b", a=AG, c=C)
                    eng.tensor_tensor(out=pd4, in0=v32g4, in1=p1b, op=ALU.mult)
                    t14 = SCR[f"t1d1{sfx}"].rearrange("p (a c b) -> p a c b", a=AG, c=8)
                    eng.tensor_tensor(out=t14, in0=pd4[:, :, 0:8, :], in1=pd4[:, :, 8:16, :], op=ALU.add)
                    t24 = SCR[f"t2d1{sfx}"].rearrange("p (a c b) -> p a c b", a=AG, c=4)
                    eng.tensor_tensor(out=t24, in0=t14[:, :, 0:4, :], in1=t14[:, :, 4:8, :], op=ALU.add)
                    t34 = SCR[f"t3d1{sfx}"].rearrange("p (a c b) -> p a c b", a=AG, c=2)
                    eng.tensor_tensor(out=t34, in0=t24[:, :, 0:2, :], in1=t24[:, :, 2:4, :], op=ALU.add)
                    eng.tensor_tensor(out=l1v[:, g * AG:(g + 1) * AG, :],
                                      in0=t34[:, :, 0, :], in1=t34[:, :, 1, :], op=ALU.add)

            def softmax(logits, layout, lsm16):
                elog = SCR["elog"]
                nc.scalar.activation(out=elog, in_=logits[:], func=ACTF.Exp)
                ssum = rt.tile([P, B], F32, tag="sqs")
                if layout == "ab":
                    ev = elog.rearrange("p (a b) -> p a b", a=A)
                    nc.vector.tensor_reduce(out=ssum[:], in_=ev.rearrange("p a b -> p b a"),
                                            axis=mybir.AxisListType.X, op=ALU.add)
                    nc.vector.reciprocal(out=ssum[:], in_=ssum[:])
                    nc.vector.tensor_tensor(
                        out=lsm16[:].rearrange("p (b a) -> p b a", b=B).rearrange("p b a -> p a b"),
                        in0=ev,
                        in1=ssum[:].unsqueeze(1).to_broadcast([P, A, B]),
                        op=ALU.mult)
                else:
                    ev = elog.rearrange("p (b a) -> p b a", b=B)
                    nc.vector.tensor_reduce(out=ssum[:], in_=ev,
                                            axis=mybir.AxisListType.X, op=ALU.add)
                    nc.vector.reciprocal(out=ssum[:], in_=ssum[:])
                    nc.vector.tensor_tensor(
                        out=lsm16[:].rearrange("p (b a) -> p b a", b=B),
                        in0=ev,
                        in1=ssum[:].unsqueeze(2).to_broadcast([P, B, A]),
                        op=ALU.mult)

            def emit_preds(v16b4, lsm16, praw):
                """praw (c,b) f32 = sum_a lsm*V ; DVE c[0:CSPL], Pool rest."""
                pr4 = SCR["prod16"].rearrange("p (c b a) -> p c b a", c=C, b=B)
                t14 = SCR["tree1"].rearrange("p (c b a) -> p c b a", c=C, b=B)
                t24 = SCR["tree2"].rearrange("p (c b a) -> p c b a", c=C, b=B)
                t34 = SCR["tree3"].rearrange("p (c b a) -> p c b a", c=C, b=B)
                t44 = SCR["tree4"].rearrange("p (c b a) -> p c b a", c=C, b=B)
                prv = praw.rearrange("p (c b) -> p c b", c=C)
                for eng, cs in ((nc.vector, slice(0, CSPL)), (nc.gpsimd, slice(CSPL, C))):
                    ncs = cs.stop - cs.start
                    lbb = lsm16[:].rearrange("p (b a) -> p b a", b=B) \
                        .unsqueeze(1).to_broadcast([P, ncs, B, A])
                    eng.tensor_tensor(out=pr4[:, cs, :, :], in0=v16b4[:, cs, :, :],
                                      in1=lbb, op=ALU.mult)
                    eng.tensor_tensor(out=t14[:, cs, :, :], in0=pr4[:, cs, :, 0:16],
                                      in1=pr4[:, cs, :, 16:32], op=ALU.add)
                    eng.tensor_tensor(out=t24[:, cs, :, :], in0=t14[:, cs, :, 0:8],
                                      in1=t14[:, cs, :, 8:16], op=ALU.add)
                    eng.tensor_tensor(out=t34[:, cs, :, :], in0=t24[:, cs, :, 0:4],
                                      in1=t24[:, cs, :, 4:8], op=ALU.add)
                    eng.tensor_tensor(out=t44[:, cs, :, :], in0=t34[:, cs, :, 0:2],
                                      in1=t34[:, cs, :, 2:4], op=ALU.add)
                    eng.tensor_tensor(out=prv[:, cs, :], in0=t44[:, cs, :, 0],
                                      in1=t44[:, cs, :, 1], op=ALU.add)

            def emit_delta2(v16b4, pcur2, lsm16, logits2):
                """logits2 (b,a) f32 = sum_c V*P2 + lsm2 ; split by b."""
                t44 = SCR["tree4"].rearrange("p (b a) -> p b a", b=B)
                t4 = SCR["tree4"]
                pr4 = SCR["prod16"].rearrange("p (c b a) -> p c b a", c=C, b=B)
                t14 = SCR["tree1"].rearrange("p (c b a) -> p c b a", c=8, b=B)
                t24 = SCR["tree2"].rearrange("p (c b a) -> p c b a", c=4, b=B)
                t34 = SCR["tree3"].rearrange("p (c b a) -> p c b a", c=2, b=B)
                p2v = pcur2.rearrange("p (c b) -> p c b", c=C)
                # Pool takes most of the (broadcast-penalized) mult in sub-slices;
                # DVE trees chase slice-by-slice.
                msl = [slice(0, 5), slice(5, 14), slice(14, 23), slice(23, 32)]
                for i, bs in enumerate(msl):
                    eng = nc.vector if i == 0 else nc.gpsimd
                    nbs = bs.stop - bs.start
                    p2b = p2v[:, :, bs].unsqueeze(3).to_broadcast([P, C, nbs, A])
                    eng.tensor_tensor(out=pr4[:, :, bs, :], in0=v16b4[:, :, bs, :],
                                      in1=p2b, op=ALU.mult)
                for bs in msl:
                    nc.vector.tensor_tensor(out=t14[:, :, bs, :], in0=pr4[:, 0:8, bs, :],
                                            in1=pr4[:, 8:16, bs, :], op=ALU.add)
                    nc.vector.tensor_tensor(out=t24[:, :, bs, :], in0=t14[:, 0:4, bs, :],
                                            in1=t14[:, 4:8, bs, :], op=ALU.add)
                    nc.vector.tensor_tensor(out=t34[:, :, bs, :], in0=t24[:, 0:2, bs, :],
                                            in1=t24[:, 2:4, bs, :], op=ALU.add)
                    nc.vector.tensor_tensor(out=t44[:, bs, :], in0=t34[:, 0, bs, :],
                                            in1=t34[:, 1, bs, :], op=ALU.add)
                nc.vector.tensor_tensor(out=logits2[:], in0=t4, in1=lsm16[:], op=ALU.add)

            # ================== main schedule ==================
            p1raw0, v16b40, v32gs0 = emit_votes(0)
            pcur1_0 = squash(p1raw0[:], "pcur1", "sq")
            logits = rt.tile([P, A * B], F32, tag="logits")
            if num_routes >= 2:
                emit_delta1(v32gs0, pcur1_0, logits, range(NG))
                nc.vector.tensor_scalar_add(out=logits[:], in0=logits[:], scalar1=1.0 / A)
            state = {"v16b4": v16b40, "pcur1": pcur1_0}

            for k in range(NCH):
                v16b4 = state["v16b4"]
                pcur1 = state["pcur1"]
                nxt = k + 1 < NCH
                p1raw_n = pcur1_n = v16b4_n = v32gs_n = None
                if nxt:
                    p1raw_n, v16b4_n, v32gs_n = emit_votes(k + 1)
                sq1_done = False

                def inject(groups, first=False):
                    nonlocal sq1_done, pcur1_n
                    if not nxt or num_routes < 2:
                        return
                    if not sq1_done:
                        pcur1_n = squash(p1raw_n[:], "pcur1", "sq")
                        sq1_done = True
                    emit_delta1(v32gs_n, pcur1_n, logits, groups)

                if num_routes == 1:
                    pcur_fin = pcur1
                    inject(range(NG))
                else:
                    layout = "ab"
                    lsm16 = rt.tile([P, B * A], F16, tag="lsm16")
                    pcur_fin = None
                    for it in range(2, num_routes + 1):
                        softmax(logits, layout, lsm16)
                        praw = SCR["praw"]
                        emit_preds(v16b4, lsm16, praw)
                        last = (it == num_routes)
                        if last:
                            inject(range(0, 3))
                        pcur = squash(praw, "pcurI", "sq")
                        if not last:
                            emit_delta2(v16b4, pcur, lsm16, logits)
                            layout = "ba"
                        else:
                            pcur_fin = pcur
                        if last:
                            inject(range(3, 6))

                nc.sync.dma_start(out=out_d.ap()[k * NCHUNK:(k + 1) * NCHUNK, :],
                                  in_=pcur_fin)

                if nxt:
                    inject(range(6, NG))
                    if num_routes >= 2:
                        nc.vector.tensor_scalar_add(out=logits[:], in0=logits[:],
                                                    scalar1=1.0 / A)
                    state = {"v16b4": v16b4_n, "pcur1": pcur1_n}

    nc.compile()
    return nc


def _prep(x, weights):
    xp = np.zeros((A, H + 2, W + 2), dtype=np.float32)
    xp[:, 1:-1, 1:-1] = x
    wvf = np.ascontiguousarray(weights.reshape(9, A, C * B))  # (c,b): idx c*B+b
    wv_h = wvf.astype(np.float16)
    wv_l = (wvf - wv_h.astype(np.float32)).astype(np.float16)
    # wp1 packed [96, 2, 3, CB]: row = (pq_local*32 + a) for q-block
    wp1 = np.empty((96, 2, 3, CB), dtype=np.float16)
    for q in range(3):
        wp1[:, 0, q, :] = wv_h[3 * q:3 * q + 3].reshape(96, CB)
        wp1[:, 1, q, :] = wv_l[3 * q:3 * q + 3].reshape(96, CB)
    in_maps = []
    for core in range(NCORES):
        r0 = core * ROWS
        pat = np.empty((9, A, ROWS, W), dtype=np.float32)
        for dp in range(3):
            for dq in range(3):
                pat[dp * 3 + dq] = xp[:, r0 + dp:r0 + dp + ROWS, dq:dq + W]
        patf = np.ascontiguousarray(pat.reshape(9, A, NPOS))
        pat_h = patf.astype(np.float16)
        pat_l = (patf - pat_h.astype(np.float32)).astype(np.float16)
        pat2 = np.stack([pat_h, pat_l], axis=2)  # [9, A, 2, NPOS]
        pp1 = np.empty((96, 2, 3, NPOS), dtype=np.float16)
        for q in range(3):
            pp1[:, 0, q, :] = pat_h[3 * q:3 * q + 3].reshape(96, NPOS)
            pp1[:, 1, q, :] = pat_l[3 * q:3 * q + 3].reshape(96, NPOS)
        in_maps.append({"pat2": np.ascontiguousarray(pat2), "wvh": wv_h, "wvl": wv_l,
                        "pp1": pp1, "wp1": wp1})
    return in_maps


def kernel(x=None, weights=None, num_routes=3, **kw):
    x = np.asarray(x, dtype=np.float32)
    weights = np.asarray(weights, dtype=np.float32)
    nr = int(num_routes)
    if nr not in _CACHE:
        _CACHE[nr] = _build_nc(nr)
    nc = _CACHE[nr]
    in_maps = _prep(x, weights)
    res = bass_utils.run_bass_kernel_spmd(nc, in_maps, core_ids=list(range(NCORES)))
    out = np.empty((B, C, H, W), dtype=np.float32)
    for core in range(NCORES):
        o = np.asarray(res.results[core]["out"]).reshape(ROWS, W, C, B)
        out[:, :, core * ROWS:(core + 1) * ROWS, :] = o.transpose(3, 2, 0, 1)
    return out


def profile_once(inputs):
    from concourse import bass_interp
    nr = int(inputs.get("num_routes", 3))
    if nr not in _CACHE:
        _CACHE[nr] = _build_nc(nr)
    sim = bass_interp.CoreSim(_CACHE[nr], no_exec=True, ignore_data_errors=True,
                              publish_trace=False)
    sim.simulate()
    return int(sim.time)


# revision 7
# speedup vs baseline: 2.2113x; 1.0032x over previous
import os, sys
import numpy as np

sys.path.insert(0, "/opt/trn_rl_repo")

from concourse import bass, bacc, bass_utils
from concourse import mybir
from concourse.tile import TileContext

F32 = mybir.dt.float32
F16 = mybir.dt.float16
ALU = mybir.AluOpType
ACTF = mybir.ActivationFunctionType

A = 32          # in_maps
B = 32          # out_maps
C = 16          # atoms
H = 64
W = 64
NCORES = 8
ROWS = H // NCORES
NPOS = ROWS * W             # 512 positions per core
NCHUNK = 128
NCH = NPOS // NCHUNK        # 4 chunks
CB = C * B                  # 512, (c,b)-order: idx = c*B + b
EPS = 1e-4
AG = 4                      # a-group size for PSUM staging
NG = A // AG                # 8 groups
P = NCHUNK

# stage split (DVE : Pool)
BSPL = 16                   # delta2 b-split: DVE gets b[0:BSPL]
CSPL = 10                   # preds c-split: DVE gets c[0:CSPL]
GSPL = 5                    # delta1 group split: DVE gets groups [0:GSPL]

_CACHE = {}


def _build_nc(num_routes: int):
    nc = bacc.Bacc(None, target_bir_lowering=False)

    pat2_d = nc.declare_dram_parameter("pat2", [9, A, 2, NPOS], F16, isOutput=False)
    wvh_d = nc.declare_dram_parameter("wvh", [9, A, CB], F16, isOutput=False)
    wvl_d = nc.declare_dram_parameter("wvl", [9, A, CB], F16, isOutput=False)
    pp1_d = nc.declare_dram_parameter("pp1", [96, 2, 3, NPOS], F16, isOutput=False)
    wp1_d = nc.declare_dram_parameter("wp1", [96, 2, 3, CB], F16, isOutput=False)
    out_d = nc.declare_dram_parameter("out", [NPOS, CB], F32, isOutput=True)

    with TileContext(nc) as tc:
        with (
            tc.tile_pool(name="wp1", bufs=1) as wp1pool,
            tc.tile_pool(name="win", bufs=2) as winpool,
            tc.tile_pool(name="v16", bufs=2) as v16pool,
            tc.tile_pool(name="v32", bufs=2) as v32pool,
            tc.tile_pool(name="psv", bufs=2, space="PSUM") as psv,
            tc.tile_pool(name="rt", bufs=1) as rt,
        ):
            # ---- static p1 weights [96, (hl q n)] one DMA
            wp1t = wp1pool.tile([96, 2 * 3 * CB], F16, name="wp1t", tag="wp1t")
            nc.sync.dma_start(out=wp1t[:].rearrange("p (h q n) -> p h q n", h=2, q=3),
                              in_=wp1_d.ap())
            wp1v = wp1t[:].rearrange("p (h q n) -> p h q n", h=2, q=3)

            # ---- resident hi weights [9, A*CB] f16 (slices loaded lazily)
            wvh = wp1pool.tile([9, A * CB], F16, name="wvh", tag="wvh")
            wvh3 = wvh[:].rearrange("q (a n) -> q a n", a=A)
            wvh_loaded = [False] * 4

            def ensure_wvh(qw):
                if not wvh_loaded[qw]:
                    asl = slice(qw * 8, (qw + 1) * 8)
                    nc.sync.dma_start(
                        out=wvh3[:, asl, :],
                        in_=wvh_d.ap()[:, asl, :])
                    wvh_loaded[qw] = True

            # ---- shared routing scratch (aliased: delta1 f32 views live in
            # scrA/scrB/tr2/tr3 whose f16 users are temporally disjoint)
            scrA = rt.tile([P, 16384], F16, tag="scrA")   # prod16 | pd1v/pd1g
            scrB = rt.tile([P, 8192], F16, tag="scrB")    # tree1  | t1d1v/g | praw/t3d1
            tr2 = rt.tile([P, 4096], F16, tag="tr2")      # tree2  | t2d1v/g | elog
            tr3 = rt.tile([P, 2048], F16, tag="tr3")      # tree3
            tr4 = rt.tile([P, 1024], F16, tag="tr4")      # tree4
            pcur1t = rt.tile([P, CB], F32, tag="pcur1t")
            SCR = {
                "prod16": scrA[:],
                "tree1": scrB[:],
                "tree2": tr2[:],
                "tree3": tr3[:],
                "tree4": tr4[:],
                "sqp2": tr4[:, 0:1024].bitcast(F32),
                "pd1v": scrA[:, 0:4096].bitcast(F32),
                "pd1g": scrA[:, 4096:8192].bitcast(F32),
                "t1d1v": scrB[:, 0:2048].bitcast(F32),
                "t1d1g": scrB[:, 2048:4096].bitcast(F32),
                "t2d1v": tr2[:, 0:1024].bitcast(F32),
                "t2d1g": tr2[:, 1024:2048].bitcast(F32),
                "t3d1v": scrB[:, 5120:5632].bitcast(F32),
                "t3d1g": scrB[:, 5632:6144].bitcast(F32),
                "elog": tr2[:, 2048:4096].bitcast(F32),
                "praw": scrB[:, 4096:5120].bitcast(F32),
                "pcur1": pcur1t[:],
            }

            def emit_votes(k):
                """DMA + PE voting + Act drains for chunk k."""
                ksl = slice(k * NCHUNK, (k + 1) * NCHUNK)
                pp1t = winpool.tile([96, 2 * 3 * NCHUNK], F16, name="pp1t", tag="pp1t", bufs=1)
                nc.sync.dma_start(
                    out=pp1t[:].rearrange("p (h q n) -> p h q n", h=2, q=3),
                    in_=pp1_d.ap()[:, :, :, ksl])
                pp1v = pp1t[:].rearrange("p (h q n) -> p h q n", h=2, q=3)
                # p1 matmuls into a shared psv-slot (uses first bank only)
                p1t = psv.tile([P, AG * CB], F32, tag="vps")
                p1ps = p1t[:, :CB]
                for q in range(3):
                    nc.tensor.matmul(out=p1ps, lhsT=pp1v[:, 0, q, :], rhs=wp1v[:, 0, q, :],
                                     start=(q == 0), stop=False)
                    nc.tensor.matmul(out=p1ps, lhsT=pp1v[:, 0, q, :], rhs=wp1v[:, 1, q, :],
                                     start=False, stop=False)
                    nc.tensor.matmul(out=p1ps, lhsT=pp1v[:, 1, q, :], rhs=wp1v[:, 0, q, :],
                                     start=False, stop=(q == 2))
                p1raw = rt.tile([P, CB], F32, tag="p1raw")
                nc.scalar.mul(out=p1raw[:], in_=p1ps, mul=1.0 / A)

                v16b = v16pool.tile([P, CB * A], F16, tag="v16b")   # (c,b,a)
                v16b4 = v16b[:].rearrange("p (c b a) -> p c b a", c=C, b=B)
                v32gs = []
                for g in range(NG):
                    w0 = g * AG
                    pkt = winpool.tile([9, AG * 2 * NCHUNK], F16, name="pk", tag="pk")
                    nc.sync.dma_start(
                        out=pkt[:].rearrange("q (a h n) -> q a h n", a=AG, h=2),
                        in_=pat2_d.ap()[:, w0:w0 + AG, :, ksl])
                    pkv = pkt[:].rearrange("q (a h n) -> q a h n", a=AG, h=2)
                    if True:
                        wlt = winpool.tile([9, AG * CB], F16, name="wgl", tag="wgl")
                        nc.sync.dma_start(
                            out=wlt[:].rearrange("q (a n) -> q a n", a=AG),
                            in_=wvl_d.ap()[:, w0:w0 + AG, :])
                        wl3 = wlt[:].rearrange("q (a n) -> q a n", a=AG)
                    ensure_wvh(g // 2)
                    vps = psv.tile([P, AG * CB], F32, tag="vps")
                    for ai in range(AG):
                        a_glob = g * AG + ai
                        aw = ai
                        osl = vps[:, ai * CB:(ai + 1) * CB]
                        nc.tensor.matmul(out=osl, lhsT=pkv[:, ai, 0, :], rhs=wvh3[:, a_glob, :],
                                         start=True, stop=False)
                        nc.tensor.matmul(out=osl, lhsT=pkv[:, ai, 0, :], rhs=wl3[:, aw, :],
                                         start=False, stop=False)
                        nc.tensor.matmul(out=osl, lhsT=pkv[:, ai, 1, :], rhs=wvh3[:, a_glob, :],
                                         start=False, stop=True)
                    for ai in range(AG):
                        a_glob = g * AG + ai
                        nc.scalar.copy(
                            out=v16b4[:, :, :, a_glob],
                            in_=vps[:, ai * CB:(ai + 1) * CB].rearrange("p (c b) -> p c b", c=C))
                    v32g = v32pool.tile([P, AG * CB], F32, tag="v32g")
                    nc.scalar.copy(out=v32g[:], in_=vps[:])
                    v32gs.append(v32g)
                return p1raw, v16b4, v32gs

            def squash(praw_cb, pcur_tag, sq_tag):
                p2 = SCR["sqp2"]
                nc.scalar.activation(out=p2, in_=praw_cb, func=ACTF.Square)
                s = rt.tile([P, B], F32, tag=f"{sq_tag}s")
                nc.vector.tensor_reduce(
                    out=s[:], in_=p2.rearrange("p (c b) -> p b c", c=C),
                    axis=mybir.AxisListType.X, op=ALU.add)
                nc.vector.tensor_scalar_add(out=s[:], in0=s[:], scalar1=EPS)  # s = sq
                nrm = rt.tile([P, B], F32, tag=f"{sq_tag}n")
                nc.scalar.activation(out=nrm[:], in_=s[:], func=ACTF.Sqrt)
                fac = rt.tile([P, B], F32, tag=f"{sq_tag}f")
                nc.vector.scalar_tensor_tensor(out=fac[:], in0=s[:], scalar=1.0,
                                               in1=nrm[:], op0=ALU.add, op1=ALU.mult)
                nc.vector.tensor_scalar_add(out=fac[:], in0=fac[:], scalar1=EPS)
                nc.vector.reciprocal(out=fac[:], in_=fac[:])
                nc.vector.tensor_tensor(out=fac[:], in0=s[:], in1=fac[:], op=ALU.mult)
                if pcur_tag == "pcur1":
                    pcur_ap = SCR["pcur1"]
                else:
                    pcur_t = rt.tile([P, CB], F32, tag=pcur_tag, name=pcur_tag)
                    pcur_ap = pcur_t[:]
                nc.vector.tensor_tensor(
                    out=pcur_ap.rearrange("p (c b) -> p c b", c=C),
                    in0=praw_cb.rearrange("p (c b) -> p c b", c=C),
                    in1=fac[:].unsqueeze(1).to_broadcast([P, C, B]),
                    op=ALU.mult)
                return pcur_ap

            def emit_delta1(v32gs, pcur1, logits1, groups):
                """logits1 (a,b) f32 = sum_c V*P1 (groups subset)."""
                l1v = logits1[:].rearrange("p (a b) -> p a b", a=A)
                p1b = pcur1.rearrange("p (c b) -> p c b", c=C) \
                    .unsqueeze(1).to_broadcast([P, AG, C, B])
                for g in groups:
                    # Pool takes the earliest groups (drained first) so it can
                    # start while DVE is still in the routing chain
                    eng = nc.gpsimd if g < (NG - GSPL) else nc.vector
                    sfx = "g" if g < (NG - GSPL) else "v"
                    v32g4 = v32gs[g][:].rearrange("p (a c b) -> p a c b", a=AG, c=C)
                    pd4 = SCR[f"pd1{sfx}"].rearrange("p (a c b) -> p a c b", a=AG, c=C)
                    eng.tensor_tensor(out=pd4, in0=v32g4, in1=p1b, op=ALU.mult)
                    t14 = SCR[f"t1d1{sfx}"].rearrange("p (a c b) -> p a c b", a=AG, c=8)
                    eng.tensor_tensor(out=t14, in0=pd4[:, :, 0:8, :], in1=pd4[:, :, 8:16, :], op=ALU.add)
                    t24 = SCR[f"t2d1{sfx}"].rearrange("p (a c b) -> p a c b", a=AG, c=4)
                    eng.tensor_tensor(out=t24, in0=t14[:, :, 0:4, :], in1=t14[:, :, 4:8, :], op=ALU.add)
                    t34 = SCR[f"t3d1{sfx}"].rearrange("p (a c b) -> p a c b", a=AG, c=2)
                    eng.tensor_tensor(out=t34, in0=t24[:, :, 0:2, :], in1=t24[:, :, 2:4, :], op=ALU.add)
                    eng.tensor_tensor(out=l1v[:, g * AG:(g + 1) * AG, :],
                                      in0=t34[:, :, 0, :], in1=t34[:, :, 1, :], op=ALU.add)

            def softmax(logits, layout, lsm16):
                elog = SCR["elog"]
                nc.scalar.activation(out=elog, in_=logits[:], func=ACTF.Exp)
                ssum = rt.tile([P, B], F32, tag="sqs")
                if layout == "ab":
                    ev = elog.rearrange("p (a b) -> p a b", a=A)
                    nc.vector.tensor_reduce(out=ssum[:], in_=ev.rearrange("p a b -> p b a"),
                                            axis=mybir.AxisListType.X, op=ALU.add)
                    nc.vector.reciprocal(out=ssum[:], in_=ssum[:])
                    nc.vector.tensor_tensor(
                        out=lsm16[:].rearrange("p (b a) -> p b a", b=B).rearrange("p b a -> p a b"),
                        in0=ev,
                        in1=ssum[:].unsqueeze(1).to_broadcast([P, A, B]),
                        op=ALU.mult)
                else:
                    ev = elog.rearrange("p (b a) -> p b a", b=B)
                    nc.vector.tensor_reduce(out=ssum[:], in_=ev,
                                            axis=mybir.AxisListType.X, op=ALU.add)
                    nc.vector.reciprocal(out=ssum[:], in_=ssum[:])
                    nc.vector.tensor_tensor(
                        out=lsm16[:].rearrange("p (b a) -> p b a", b=B),
                        in0=ev,
                        in1=ssum[:].unsqueeze(2).to_broadcast([P, B, A]),
                        op=ALU.mult)

            def emit_preds(v16b4, lsm16, praw):
                """praw (c,b) f32 = sum_a lsm*V ; DVE c[0:CSPL], Pool rest."""
                pr4 = SCR["prod16"].rearrange("p (c b a) -> p c b a", c=C, b=B)
                t14 = SCR["tree1"].rearrange("p (c b a) -> p c b a", c=C, b=B)
                t24 = SCR["tree2"].rearrange("p (c b a) -> p c b a", c=C, b=B)
                t34 = SCR["tree3"].rearrange("p (c b a) -> p c b a", c=C, b=B)
                t44 = SCR["tree4"].rearrange("p (c b a) -> p c b a", c=C, b=B)
                prv = praw.rearrange("p (c b) -> p c b", c=C)
                for eng, cs in ((nc.vector, slice(0, CSPL)), (nc.gpsimd, slice(CSPL, C))):
                    ncs = cs.stop - cs.start
                    lbb = lsm16[:].rearrange("p (b a) -> p b a", b=B) \
                        .unsqueeze(1).to_broadcast([P, ncs, B, A])
                    eng.tensor_tensor(out=pr4[:, cs, :, :], in0=v16b4[:, cs, :, :],
                                      in1=lbb, op=ALU.mult)
                    eng.tensor_tensor(out=t14[:, cs, :, :], in0=pr4[:, cs, :, 0:16],
                                      in1=pr4[:, cs, :, 16:32], op=ALU.add)
                    eng.tensor_tensor(out=t24[:, cs, :, :], in0=t14[:, cs, :, 0:8],
                                      in1=t14[:, cs, :, 8:16], op=ALU.add)
                    eng.tensor_tensor(out=t34[:, cs, :, :], in0=t24[:, cs, :, 0:4],
                                      in1=t24[:, cs, :, 4:8], op=ALU.add)
                    eng.tensor_tensor(out=t44[:, cs, :, :], in0=t34[:, cs, :, 0:2],
                                      in1=t34[:, cs, :, 2:4], op=ALU.add)
                    eng.tensor_tensor(out=prv[:, cs, :], in0=t44[:, cs, :, 0],
                                      in1=t44[:, cs, :, 1], op=ALU.add)

            def emit_delta2(v16b4, pcur2, lsm16, logits2):
                """logits2 (b,a) f32 = sum_c V*P2 + lsm2 ; split by b."""
                t44 = SCR["tree4"].rearrange("p (b a) -> p b a", b=B)
                t4 = SCR["tree4"]
                pr4 = SCR["prod16"].rearrange("p (c b a) -> p c b a", c=C, b=B)
                t14 = SCR["tree1"].rearrange("p (c b a) -> p c b a", c=8, b=B)
                t24 = SCR["tree2"].rearrange("p (c b a) -> p c b a", c=4, b=B)
                t34 = SCR["tree3"].rearrange("p (c b a) -> p c b a", c=2, b=B)
                p2v = pcur2.rearrange("p (c b) -> p c b", c=C)
                # Pool takes most of the (broadcast-penalized) mult in sub-slices;
                # DVE trees chase slice-by-slice.
                msl = [slice(0, 5), slice(5, 14), slice(14, 23), slice(23, 32)]
                for i, bs in enumerate(msl):
                    eng = nc.vector if i == 0 else nc.gpsimd
                    nbs = bs.stop - bs.start
                    p2b = p2v[:, :, bs].unsqueeze(3).to_broadcast([P, C, nbs, A])
                    eng.tensor_tensor(out=pr4[:, :, bs, :], in0=v16b4[:, :, bs, :],
                                      in1=p2b, op=ALU.mult)
                for bs in msl:
                    nc.vector.tensor_tensor(out=t14[:, :, bs, :], in0=pr4[:, 0:8, bs, :],
                                            in1=pr4[:, 8:16, bs, :], op=ALU.add)
                    nc.vector.tensor_tensor(out=t24[:, :, bs, :], in0=t14[:, 0:4, bs, :],
                                            in1=t14[:, 4:8, bs, :], op=ALU.add)
                    nc.vector.tensor_tensor(out=t34[:, :, bs, :], in0=t24[:, 0:2, bs, :],
                                            in1=t24[:, 2:4, bs, :], op=ALU.add)
                    nc.vector.tensor_tensor(out=t44[:, bs, :], in0=t34[:, 0, bs, :],
                                            in1=t34[:, 1, bs, :], op=ALU.add)
                nc.vector.tensor_tensor(out=logits2[:], in0=t4, in1=lsm16[:], op=ALU.add)

            # ================== main schedule ==================
            p1raw0, v16b40, v32gs0 = emit_votes(0)
            pcur1_0 = squash(p1raw0[:], "pcur1", "sq")
            logits = rt.tile([P, A * B], F32, tag="logits")
            if num_routes >= 2:
                emit_delta1(v32gs0, pcur1_0, logits, range(NG))
                nc.vector.tensor_scalar_add(out=logits[:], in0=logits[:], scalar1=1.0 / A)
            state = {"v16b4": v16b40, "pcur1": pcur1_0}

            for k in range(NCH):
                v16b4 = state["v16b4"]
                pcur1 = state["pcur1"]
                nxt = k + 1 < NCH
                p1raw_n = pcur1_n = v16b4_n = v32gs_n = None
                if nxt:
                    p1raw_n, v16b4_n, v32gs_n = emit_votes(k + 1)
                sq1_done = False

                def inject(groups, first=False):
                    nonlocal sq1_done, pcur1_n
                    if not nxt or num_routes < 2:
                        return
                    if not sq1_done:
                        pcur1_n = squash(p1raw_n[:], "pcur1", "sq")
                        sq1_done = True
                    emit_delta1(v32gs_n, pcur1_n, logits, groups)

                if num_routes == 1:
                    pcur_fin = pcur1
                    inject(range(NG))
                else:
                    layout = "ab"
                    lsm16 = rt.tile([P, B * A], F16, tag="lsm16")
                    pcur_fin = None
                    for it in range(2, num_routes + 1):
                        softmax(logits, layout, lsm16)
                        praw = SCR["praw"]
                        emit_preds(v16b4, lsm16, praw)
                        last = (it == num_routes)
                        if last:
                            inject(range(0, 3))
                        pcur = squash(praw, "pcurI", "sq")
                        if not last:
                            emit_delta2(v16b4, pcur, lsm16, logits)
                            layout = "ba"
                        else:
                            pcur_fin = pcur
                        if last:
                            inject(range(3, 6))

                nc.sync.dma_start(out=out_d.ap()[k * NCHUNK:(k + 1) * NCHUNK, :],
                                  in_=pcur_fin)

                if nxt:
                    inject(range(6, NG))
                    if num_routes >= 2:
                        nc.vector.tensor_scalar_add(out=logits[:], in0=logits[:],
                                                    scalar1=1.0 / A)
                    state = {"v16b4": v16b4_n, "pcur1": pcur1_n}

    nc.compile()
    return nc


def _prep(x, weights):
    xp = np.zeros((A, H + 2, W + 2), dtype=np.float32)
    xp[:, 1:-1, 1:-1] = x
    wvf = np.ascontiguousarray(weights.reshape(9, A, C * B))  # (c,b): idx c*B+b
    wv_h = wvf.astype(np.float16)
    wv_l = (wvf - wv_h.astype(np.float32)).astype(np.float16)
    # wp1 packed [96, 2, 3, CB]: row = (pq_local*32 + a) for q-block
    wp1 = np.empty((96, 2, 3, CB), dtype=np.float16)
    for q in range(3):
        wp1[:, 0, q, :] = wv_h[3 * q:3 * q + 3].reshape(96, CB)
        wp1[:, 1, q, :] = wv_l[3 * q:3 * q + 3].reshape(96, CB)
    in_maps = []
    for core in range(NCORES):
        r0 = core * ROWS
        pat = np.empty((9, A, ROWS, W), dtype=np.float32)
        for dp in range(3):
            for dq in range(3):
                pat[dp * 3 + dq] = xp[:, r0 + dp:r0 + dp + ROWS, dq:dq + W]
        patf = np.ascontiguousarray(pat.reshape(9, A, NPOS))
        pat_h = patf.astype(np.float16)
        pat_l = (patf - pat_h.astype(np.float32)).astype(np.float16)
        pat2 = np.stack([pat_h, pat_l], axis=2)  # [9, A, 2, NPOS]
        pp1 = np.empty((96, 2, 3, NPOS), dtype=np.float16)
        for q in range(3):
            pp1[:, 0, q, :] = pat_h[3 * q:3 * q + 3].reshape(96, NPOS)
            pp1[:, 1, q, :] = pat_l[3 * q:3 * q + 3].reshape(96, NPOS)
        in_maps.append({"pat2": np.ascontiguousarray(pat2), "wvh": wv_h, "wvl": wv_l,
                        "pp1": pp1, "wp1": wp1})
    return in_maps


def kernel(x=None, weights=None, num_routes=3, **kw):
    x = np.asarray(x, dtype=np.float32)
    weights = np.asarray(weights, dtype=np.float32)
    nr = int(num_routes)
    if nr not in _CACHE:
        _CACHE[nr] = _build_nc(nr)
    nc = _CACHE[nr]
    in_maps = _prep(x, weights)
    res = bass_utils.run_bass_kernel_spmd(nc, in_maps, core_ids=list(range(NCORES)))
    out = np.empty((B, C, H, W), dtype=np.float32)
    for core in range(NCORES):
        o = np.asarray(res.results[core]["out"]).reshape(ROWS, W, C, B)
        out[:, :, core * ROWS:(core + 1) * ROWS, :] = o.transpose(3, 2, 0, 1)
    return out


def profile_once(inputs):
    from concourse import bass_interp
    nr = int(inputs.get("num_routes", 3))
    if nr not in _CACHE:
        _CACHE[nr] = _build_nc(nr)
    sim = bass_interp.CoreSim(_CACHE[nr], no_exec=True, ignore_data_errors=True,
                              publish_trace=False)
    sim.simulate()
    return int(sim.time)


# revision 8
# speedup vs baseline: 2.2198x; 1.0038x over previous
import os, sys
import numpy as np

sys.path.insert(0, "/opt/trn_rl_repo")

from concourse import bass, bacc, bass_utils
from concourse import mybir
from concourse.tile import TileContext

F32 = mybir.dt.float32
F16 = mybir.dt.float16
ALU = mybir.AluOpType
ACTF = mybir.ActivationFunctionType

A = 32          # in_maps
B = 32          # out_maps
C = 16          # atoms
H = 64
W = 64
NCORES = 8
ROWS = H // NCORES
NPOS = ROWS * W             # 512 positions per core
NCHUNK = 128
NCH = NPOS // NCHUNK        # 4 chunks
CB = C * B                  # 512, (c,b)-order: idx = c*B + b
EPS = 1e-4
AG = 4                      # a-group size for PSUM staging
NG = A // AG                # 8 groups
P = NCHUNK

# stage split (DVE : Pool)
BSPL = 16                   # delta2 b-split: DVE gets b[0:BSPL]
CSPL = 10                   # preds c-split: DVE gets c[0:CSPL]
GSPL = 5                    # delta1 group split: DVE gets groups [0:GSPL]

_CACHE = {}


def _build_nc(num_routes: int):
    nc = bacc.Bacc(None, target_bir_lowering=False)

    pat2_d = nc.declare_dram_parameter("pat2", [9, A, 2, NPOS], F16, isOutput=False)
    wvh_d = nc.declare_dram_parameter("wvh", [9, A, CB], F16, isOutput=False)
    wvl_d = nc.declare_dram_parameter("wvl", [9, A, CB], F16, isOutput=False)
    pp1_d = nc.declare_dram_parameter("pp1", [96, 2, 3, NPOS], F16, isOutput=False)
    wp1_d = nc.declare_dram_parameter("wp1", [96, 2, 3, CB], F16, isOutput=False)
    out_d = nc.declare_dram_parameter("out", [NPOS, CB], F32, isOutput=True)

    with TileContext(nc) as tc:
        with (
            tc.tile_pool(name="wp1", bufs=1) as wp1pool,
            tc.tile_pool(name="win", bufs=2) as winpool,
            tc.tile_pool(name="v16", bufs=2) as v16pool,
            tc.tile_pool(name="v32", bufs=2) as v32pool,
            tc.tile_pool(name="psv", bufs=2, space="PSUM") as psv,
            tc.tile_pool(name="rt", bufs=1) as rt,
        ):
            # ---- static p1 weights [96, (hl q n)] one DMA
            wp1t = wp1pool.tile([96, 2 * 3 * CB], F16, name="wp1t", tag="wp1t")
            nc.sync.dma_start(out=wp1t[:].rearrange("p (h q n) -> p h q n", h=2, q=3),
                              in_=wp1_d.ap())
            wp1v = wp1t[:].rearrange("p (h q n) -> p h q n", h=2, q=3)

            # ---- resident hi weights [9, A*CB] f16 (slices loaded lazily)
            wvh = wp1pool.tile([9, A * CB], F16, name="wvh", tag="wvh")
            wvh3 = wvh[:].rearrange("q (a n) -> q a n", a=A)
            wvh_loaded = [False] * 4

            def ensure_wvh(qw):
                if not wvh_loaded[qw]:
                    asl = slice(qw * 8, (qw + 1) * 8)
                    nc.sync.dma_start(
                        out=wvh3[:, asl, :],
                        in_=wvh_d.ap()[:, asl, :])
                    wvh_loaded[qw] = True

            # ---- shared routing scratch (aliased: delta1 f32 views live in
            # scrA/scrB/tr2/tr3 whose f16 users are temporally disjoint)
            scrA = rt.tile([P, 16384], F16, tag="scrA")   # prod16 | pd1v/pd1g
            scrB = rt.tile([P, 8192], F16, tag="scrB")    # tree1  | t1d1v/g | praw/t3d1
            tr2 = rt.tile([P, 4096], F16, tag="tr2")      # tree2  | t2d1v/g | elog
            tr3 = rt.tile([P, 2048], F16, tag="tr3")      # tree3
            tr4 = rt.tile([P, 1024], F16, tag="tr4")      # tree4
            pcur1t = rt.tile([P, CB], F32, tag="pcur1t")
            SCR = {
                "prod16": scrA[:],
                "tree1": scrB[:],
                "tree2": tr2[:],
                "tree3": tr3[:],
                "tree4": tr4[:],
                "sqp2": tr4[:, 0:1024].bitcast(F32),
                "pd1v": scrA[:, 0:4096].bitcast(F32),
                "pd1g": scrA[:, 4096:8192].bitcast(F32),
                "t1d1v": scrB[:, 0:2048].bitcast(F32),
                "t1d1g": scrB[:, 2048:4096].bitcast(F32),
                "t2d1v": tr2[:, 0:1024].bitcast(F32),
                "t2d1g": tr2[:, 1024:2048].bitcast(F32),
                "t3d1v": scrB[:, 5120:5632].bitcast(F32),
                "t3d1g": scrB[:, 5632:6144].bitcast(F32),
                "elog": tr2[:, 2048:4096].bitcast(F32),
                "praw": scrB[:, 4096:5120].bitcast(F32),
                "pcur1": pcur1t[:],
            }

            def emit_votes(k):
                """DMA + PE voting + Act drains for chunk k."""
                ksl = slice(k * NCHUNK, (k + 1) * NCHUNK)
                pp1t = winpool.tile([96, 2 * 3 * NCHUNK], F16, name="pp1t", tag="pp1t", bufs=1)
                nc.sync.dma_start(
                    out=pp1t[:].rearrange("p (h q n) -> p h q n", h=2, q=3),
                    in_=pp1_d.ap()[:, :, :, ksl])
                pp1v = pp1t[:].rearrange("p (h q n) -> p h q n", h=2, q=3)
                # p1 matmuls into a shared psv-slot (uses first bank only)
                p1t = psv.tile([P, AG * CB], F32, tag="vps")
                p1ps = p1t[:, :CB]
                for q in range(3):
                    nc.tensor.matmul(out=p1ps, lhsT=pp1v[:, 0, q, :], rhs=wp1v[:, 0, q, :],
                                     start=(q == 0), stop=False)
                    nc.tensor.matmul(out=p1ps, lhsT=pp1v[:, 0, q, :], rhs=wp1v[:, 1, q, :],
                                     start=False, stop=False)
                    nc.tensor.matmul(out=p1ps, lhsT=pp1v[:, 1, q, :], rhs=wp1v[:, 0, q, :],
                                     start=False, stop=(q == 2))
                p1raw = rt.tile([P, CB], F32, tag="p1raw")
                nc.scalar.mul(out=p1raw[:], in_=p1ps, mul=1.0 / A)

                v16b = v16pool.tile([P, CB * A], F16, tag="v16b")   # (c,b,a)
                v16b4 = v16b[:].rearrange("p (c b a) -> p c b a", c=C, b=B)
                v32gs = []
                for g in range(NG):
                    w0 = g * AG
                    pkt = winpool.tile([9, AG * 2 * NCHUNK], F16, name="pk", tag="pk")
                    nc.sync.dma_start(
                        out=pkt[:].rearrange("q (a h n) -> q a h n", a=AG, h=2),
                        in_=pat2_d.ap()[:, w0:w0 + AG, :, ksl])
                    pkv = pkt[:].rearrange("q (a h n) -> q a h n", a=AG, h=2)
                    if True:
                        wlt = winpool.tile([9, AG * CB], F16, name="wgl", tag="wgl")
                        nc.sync.dma_start(
                            out=wlt[:].rearrange("q (a n) -> q a n", a=AG),
                            in_=wvl_d.ap()[:, w0:w0 + AG, :])
                        wl3 = wlt[:].rearrange("q (a n) -> q a n", a=AG)
                    ensure_wvh(g // 2)
                    vps = psv.tile([P, AG * CB], F32, tag="vps")
                    for ai in range(AG):
                        a_glob = g * AG + ai
                        aw = ai
                        osl = vps[:, ai * CB:(ai + 1) * CB]
                        nc.tensor.matmul(out=osl, lhsT=pkv[:, ai, 0, :], rhs=wvh3[:, a_glob, :],
                                         start=True, stop=False)
                        nc.tensor.matmul(out=osl, lhsT=pkv[:, ai, 0, :], rhs=wl3[:, aw, :],
                                         start=False, stop=False)
                        nc.tensor.matmul(out=osl, lhsT=pkv[:, ai, 1, :], rhs=wvh3[:, a_glob, :],
                                         start=False, stop=True)
                    for ai in range(AG):
                        a_glob = g * AG + ai
                        nc.scalar.copy(
                            out=v16b4[:, :, :, a_glob],
                            in_=vps[:, ai * CB:(ai + 1) * CB].rearrange("p (c b) -> p c b", c=C))
                    v32g = v32pool.tile([P, AG * CB], F32, tag="v32g")
                    nc.scalar.copy(out=v32g[:], in_=vps[:])
                    v32gs.append(v32g)
                return p1raw, v16b4, v32gs

            def squash(praw_cb, pcur_tag, sq_tag):
                p2 = SCR["sqp2"]
                nc.gpsimd.tensor_tensor(out=p2, in0=praw_cb, in1=praw_cb, op=ALU.mult)
                s = rt.tile([P, B], F32, tag=f"{sq_tag}s")
                nc.vector.tensor_reduce(
                    out=s[:], in_=p2.rearrange("p (c b) -> p b c", c=C),
                    axis=mybir.AxisListType.X, op=ALU.add)
                nc.vector.tensor_scalar_add(out=s[:], in0=s[:], scalar1=EPS)  # s = sq
                nrm = rt.tile([P, B], F32, tag=f"{sq_tag}n")
                nc.scalar.activation(out=nrm[:], in_=s[:], func=ACTF.Sqrt)
                fac = rt.tile([P, B], F32, tag=f"{sq_tag}f")
                nc.vector.scalar_tensor_tensor(out=fac[:], in0=s[:], scalar=1.0,
                                               in1=nrm[:], op0=ALU.add, op1=ALU.mult)
                nc.vector.tensor_scalar_add(out=fac[:], in0=fac[:], scalar1=EPS)
                nc.vector.reciprocal(out=fac[:], in_=fac[:])
                nc.vector.tensor_tensor(out=fac[:], in0=s[:], in1=fac[:], op=ALU.mult)
                if pcur_tag == "pcur1":
                    pcur_ap = SCR["pcur1"]
                else:
                    pcur_t = rt.tile([P, CB], F32, tag=pcur_tag, name=pcur_tag)
                    pcur_ap = pcur_t[:]
                nc.vector.tensor_tensor(
                    out=pcur_ap.rearrange("p (c b) -> p c b", c=C),
                    in0=praw_cb.rearrange("p (c b) -> p c b", c=C),
                    in1=fac[:].unsqueeze(1).to_broadcast([P, C, B]),
                    op=ALU.mult)
                return pcur_ap

            def emit_delta1(v32gs, pcur1, logits1, groups):
                """logits1 (a,b) f32 = sum_c V*P1 (groups subset)."""
                l1v = logits1[:].rearrange("p (a b) -> p a b", a=A)
                p1b = pcur1.rearrange("p (c b) -> p c b", c=C) \
                    .unsqueeze(1).to_broadcast([P, AG, C, B])
                for g in groups:
                    # Pool takes the earliest groups (drained first) so it can
                    # start while DVE is still in the routing chain
                    eng = nc.gpsimd if g < (NG - GSPL) else nc.vector
                    sfx = "g" if g < (NG - GSPL) else "v"
                    v32g4 = v32gs[g][:].rearrange("p (a c b) -> p a c b", a=AG, c=C)
                    pd4 = SCR[f"pd1{sfx}"].rearrange("p (a c b) -> p a c b", a=AG, c=C)
                    eng.tensor_tensor(out=pd4, in0=v32g4, in1=p1b, op=ALU.mult)
                    t14 = SCR[f"t1d1{sfx}"].rearrange("p (a c b) -> p a c b", a=AG, c=8)
                    eng.tensor_tensor(out=t14, in0=pd4[:, :, 0:8, :], in1=pd4[:, :, 8:16, :], op=ALU.add)
                    t24 = SCR[f"t2d1{sfx}"].rearrange("p (a c b) -> p a c b", a=AG, c=4)
                    eng.tensor_tensor(out=t24, in0=t14[:, :, 0:4, :], in1=t14[:, :, 4:8, :], op=ALU.add)
                    t34 = SCR[f"t3d1{sfx}"].rearrange("p (a c b) -> p a c b", a=AG, c=2)
                    eng.tensor_tensor(out=t34, in0=t24[:, :, 0:2, :], in1=t24[:, :, 2:4, :], op=ALU.add)
                    eng.tensor_tensor(out=l1v[:, g * AG:(g + 1) * AG, :],
                                      in0=t34[:, :, 0, :], in1=t34[:, :, 1, :], op=ALU.add)

            def softmax(logits, layout, lsm16):
                elog = SCR["elog"]
                nc.scalar.activation(out=elog, in_=logits[:], func=ACTF.Exp)
                ssum = rt.tile([P, B], F32, tag="sqs")
                if layout == "ab":
                    ev = elog.rearrange("p (a b) -> p a b", a=A)
                    nc.vector.tensor_reduce(out=ssum[:], in_=ev.rearrange("p a b -> p b a"),
                                            axis=mybir.AxisListType.X, op=ALU.add)
                    nc.vector.reciprocal(out=ssum[:], in_=ssum[:])
                    nc.vector.tensor_tensor(
                        out=lsm16[:].rearrange("p (b a) -> p b a", b=B).rearrange("p b a -> p a b"),
                        in0=ev,
                        in1=ssum[:].unsqueeze(1).to_broadcast([P, A, B]),
                        op=ALU.mult)
                else:
                    ev = elog.rearrange("p (b a) -> p b a", b=B)
                    nc.vector.tensor_reduce(out=ssum[:], in_=ev,
                                            axis=mybir.AxisListType.X, op=ALU.add)
                    nc.vector.reciprocal(out=ssum[:], in_=ssum[:])
                    nc.vector.tensor_tensor(
                        out=lsm16[:].rearrange("p (b a) -> p b a", b=B),
                        in0=ev,
                        in1=ssum[:].unsqueeze(2).to_broadcast([P, B, A]),
                        op=ALU.mult)

            def emit_preds(v16b4, lsm16, praw):
                """praw (c,b) f32 = sum_a lsm*V ; DVE c[0:CSPL], Pool rest."""
                pr4 = SCR["prod16"].rearrange("p (c b a) -> p c b a", c=C, b=B)
                t14 = SCR["tree1"].rearrange("p (c b a) -> p c b a", c=C, b=B)
                t24 = SCR["tree2"].rearrange("p (c b a) -> p c b a", c=C, b=B)
                t34 = SCR["tree3"].rearrange("p (c b a) -> p c b a", c=C, b=B)
                t44 = SCR["tree4"].rearrange("p (c b a) -> p c b a", c=C, b=B)
                prv = praw.rearrange("p (c b) -> p c b", c=C)
                for eng, cs in ((nc.vector, slice(0, CSPL)), (nc.gpsimd, slice(CSPL, C))):
                    ncs = cs.stop - cs.start
                    lbb = lsm16[:].rearrange("p (b a) -> p b a", b=B) \
                        .unsqueeze(1).to_broadcast([P, ncs, B, A])
                    eng.tensor_tensor(out=pr4[:, cs, :, :], in0=v16b4[:, cs, :, :],
                                      in1=lbb, op=ALU.mult)
                    eng.tensor_tensor(out=t14[:, cs, :, :], in0=pr4[:, cs, :, 0:16],
                                      in1=pr4[:, cs, :, 16:32], op=ALU.add)
                    eng.tensor_tensor(out=t24[:, cs, :, :], in0=t14[:, cs, :, 0:8],
                                      in1=t14[:, cs, :, 8:16], op=ALU.add)
                    eng.tensor_tensor(out=t34[:, cs, :, :], in0=t24[:, cs, :, 0:4],
                                      in1=t24[:, cs, :, 4:8], op=ALU.add)
                    eng.tensor_tensor(out=t44[:, cs, :, :], in0=t34[:, cs, :, 0:2],
                                      in1=t34[:, cs, :, 2:4], op=ALU.add)
                    eng.tensor_tensor(out=prv[:, cs, :], in0=t44[:, cs, :, 0],
                                      in1=t44[:, cs, :, 1], op=ALU.add)

            def emit_delta2(v16b4, pcur2, lsm16, logits2):
                """logits2 (b,a) f32 = sum_c V*P2 + lsm2 ; split by b."""
                t44 = SCR["tree4"].rearrange("p (b a) -> p b a", b=B)
                t4 = SCR["tree4"]
                pr4 = SCR["prod16"].rearrange("p (c b a) -> p c b a", c=C, b=B)
                t14 = SCR["tree1"].rearrange("p (c b a) -> p c b a", c=8, b=B)
                t24 = SCR["tree2"].rearrange("p (c b a) -> p c b a", c=4, b=B)
                t34 = SCR["tree3"].rearrange("p (c b a) -> p c b a", c=2, b=B)
                p2v = pcur2.rearrange("p (c b) -> p c b", c=C)
                # Pool takes most of the (broadcast-penalized) mult in sub-slices;
                # DVE trees chase slice-by-slice.
                msl = [slice(0, 5), slice(5, 14), slice(14, 23), slice(23, 32)]
                for i, bs in enumerate(msl):
                    eng = nc.vector if i == 0 else nc.gpsimd
                    nbs = bs.stop - bs.start
                    p2b = p2v[:, :, bs].unsqueeze(3).to_broadcast([P, C, nbs, A])
                    eng.tensor_tensor(out=pr4[:, :, bs, :], in0=v16b4[:, :, bs, :],
                                      in1=p2b, op=ALU.mult)
                for bs in msl:
                    nc.vector.tensor_tensor(out=t14[:, :, bs, :], in0=pr4[:, 0:8, bs, :],
                                            in1=pr4[:, 8:16, bs, :], op=ALU.add)
                    nc.vector.tensor_tensor(out=t24[:, :, bs, :], in0=t14[:, 0:4, bs, :],
                                            in1=t14[:, 4:8, bs, :], op=ALU.add)
                    nc.vector.tensor_tensor(out=t34[:, :, bs, :], in0=t24[:, 0:2, bs, :],
                                            in1=t24[:, 2:4, bs, :], op=ALU.add)
                    nc.vector.tensor_tensor(out=t44[:, bs, :], in0=t34[:, 0, bs, :],
                                            in1=t34[:, 1, bs, :], op=ALU.add)
                nc.vector.tensor_tensor(out=logits2[:], in0=t4, in1=lsm16[:], op=ALU.add)

            # ================== main schedule ==================
            p1raw0, v16b40, v32gs0 = emit_votes(0)
            pcur1_0 = squash(p1raw0[:], "pcur1", "sq")
            logits = rt.tile([P, A * B], F32, tag="logits")
            if num_routes >= 2:
                emit_delta1(v32gs0, pcur1_0, logits, range(NG))
                nc.vector.tensor_scalar_add(out=logits[:], in0=logits[:], scalar1=1.0 / A)
            state = {"v16b4": v16b40, "pcur1": pcur1_0}

            for k in range(NCH):
                v16b4 = state["v16b4"]
                pcur1 = state["pcur1"]
                nxt = k + 1 < NCH
                p1raw_n = pcur1_n = v16b4_n = v32gs_n = None
                if nxt:
                    p1raw_n, v16b4_n, v32gs_n = emit_votes(k + 1)
                sq1_done = False

                def inject(groups, first=False):
                    nonlocal sq1_done, pcur1_n
                    if not nxt or num_routes < 2:
                        return
                    if not sq1_done:
                        pcur1_n = squash(p1raw_n[:], "pcur1", "sq")
                        sq1_done = True
                    emit_delta1(v32gs_n, pcur1_n, logits, groups)

                if num_routes == 1:
                    pcur_fin = pcur1
                    inject(range(NG))
                else:
                    layout = "ab"
                    lsm16 = rt.tile([P, B * A], F16, tag="lsm16")
                    pcur_fin = None
                    for it in range(2, num_routes + 1):
                        softmax(logits, layout, lsm16)
                        praw = SCR["praw"]
                        emit_preds(v16b4, lsm16, praw)
                        last = (it == num_routes)
                        if last:
                            inject(range(0, 3))
                        pcur = squash(praw, "pcurI", "sq")
                        if not last:
                            emit_delta2(v16b4, pcur, lsm16, logits)
                            layout = "ba"
                        else:
                            pcur_fin = pcur
                        if last:
                            inject(range(3, 6))

                nc.sync.dma_start(out=out_d.ap()[k * NCHUNK:(k + 1) * NCHUNK, :],
                                  in_=pcur_fin)

                if nxt:
                    inject(range(6, NG))
                    if num_routes >= 2:
                        nc.vector.tensor_scalar_add(out=logits[:], in0=logits[:],
                                                    scalar1=1.0 / A)
                    state = {"v16b4": v16b4_n, "pcur1": pcur1_n}

    nc.compile()
    return nc


def _prep(x, weights):
    xp = np.zeros((A, H + 2, W + 2), dtype=np.float32)
    xp[:, 1:-1, 1:-1] = x
    wvf = np.ascontiguousarray(weights.reshape(9, A, C * B))  # (c,b): idx c*B+b
    wv_h = wvf.astype(np.float16)
    wv_l = (wvf - wv_h.astype(np.float32)).astype(np.float16)
    # wp1 packed [96, 2, 3, CB]: row = (pq_local*32 + a) for q-block
    wp1 = np.empty((96, 2, 3, CB), dtype=np.float16)
    for q in range(3):
        wp1[:, 0, q, :] = wv_h[3 * q:3 * q + 3].reshape(96, CB)
        wp1[:, 1, q, :] = wv_l[3 * q:3 * q + 3].reshape(96, CB)
    in_maps = []
    for core in range(NCORES):
        r0 = core * ROWS
        pat = np.empty((9, A, ROWS, W), dtype=np.float32)
        for dp in range(3):
            for dq in range(3):
                pat[dp * 3 + dq] = xp[:, r0 + dp:r0 + dp + ROWS, dq:dq + W]
        patf = np.ascontiguousarray(pat.reshape(9, A, NPOS))
        pat_h = patf.astype(np.float16)
        pat_l = (patf - pat_h.astype(np.float32)).astype(np.float16)
        pat2 = np.stack([pat_h, pat_l], axis=2)  # [9, A, 2, NPOS]
        pp1 = np.empty((96, 2, 3, NPOS), dtype=np.float16)
        for q in range(3):
            pp1[:, 0, q, :] = pat_h[3 * q:3 * q + 3].reshape(96, NPOS)
            pp1[:, 1, q, :] = pat_l[3 * q:3 * q + 3].reshape(96, NPOS)
        in_maps.append({"pat2": np.ascontiguousarray(pat2), "wvh": wv_h, "wvl": wv_l,
                        "pp1": pp1, "wp1": wp1})
    return in_maps


def kernel(x=None, weights=None, num_routes=3, **kw):
    x = np.asarray(x, dtype=np.float32)
    weights = np.asarray(weights, dtype=np.float32)
    nr = int(num_routes)
    if nr not in _CACHE:
        _CACHE[nr] = _build_nc(nr)
    nc = _CACHE[nr]
    in_maps = _prep(x, weights)
    res = bass_utils.run_bass_kernel_spmd(nc, in_maps, core_ids=list(range(NCORES)))
    out = np.empty((B, C, H, W), dtype=np.float32)
    for core in range(NCORES):
        o = np.asarray(res.results[core]["out"]).reshape(ROWS, W, C, B)
        out[:, :, core * ROWS:(core + 1) * ROWS, :] = o.transpose(3, 2, 0, 1)
    return out


def profile_once(inputs):
    from concourse import bass_interp
    nr = int(inputs.get("num_routes", 3))
    if nr not in _CACHE:
        _CACHE[nr] = _build_nc(nr)
    sim = bass_interp.CoreSim(_CACHE[nr], no_exec=True, ignore_data_errors=True,
                              publish_trace=False)
    sim.simulate()
    return int(sim.time)


# revision 9
# speedup vs baseline: 2.2270x; 1.0033x over previous
import os, sys
import numpy as np

sys.path.insert(0, "/opt/trn_rl_repo")

from concourse import bass, bacc, bass_utils
from concourse import mybir
from concourse.tile import TileContext

F32 = mybir.dt.float32
F16 = mybir.dt.float16
ALU = mybir.AluOpType
ACTF = mybir.ActivationFunctionType

A = 32          # in_maps
B = 32          # out_maps
C = 16          # atoms
H = 64
W = 64
NCORES = 8
ROWS = H // NCORES
NPOS = ROWS * W             # 512 positions per core
NCHUNK = 128
NCH = NPOS // NCHUNK        # 4 chunks
CB = C * B                  # 512, (c,b)-order: idx = c*B + b
EPS = 1e-4
AG = 4                      # a-group size for PSUM staging
NG = A // AG                # 8 groups
P = NCHUNK

# stage split (DVE : Pool)
BSPL = 16                   # delta2 b-split: DVE gets b[0:BSPL]
CSPL = 10                   # preds c-split: DVE gets c[0:CSPL]
GSPL = 5                    # delta1 group split: DVE gets groups [0:GSPL]

_CACHE = {}


def _build_nc(num_routes: int):
    nc = bacc.Bacc(None, target_bir_lowering=False)

    pat2_d = nc.declare_dram_parameter("pat2", [9, A, 2, NPOS], F16, isOutput=False)
    wvh_d = nc.declare_dram_parameter("wvh", [9, A, CB], F16, isOutput=False)
    wvl_d = nc.declare_dram_parameter("wvl", [9, A, CB], F16, isOutput=False)
    pp1_d = nc.declare_dram_parameter("pp1", [96, 2, 3, NPOS], F16, isOutput=False)
    wp1_d = nc.declare_dram_parameter("wp1", [96, 2, 3, CB], F16, isOutput=False)
    out_d = nc.declare_dram_parameter("out", [NPOS, CB], F32, isOutput=True)

    with TileContext(nc) as tc:
        with (
            tc.tile_pool(name="wp1", bufs=1) as wp1pool,
            tc.tile_pool(name="win", bufs=2) as winpool,
            tc.tile_pool(name="v16", bufs=2) as v16pool,
            tc.tile_pool(name="v32", bufs=2) as v32pool,
            tc.tile_pool(name="psv", bufs=2, space="PSUM") as psv,
            tc.tile_pool(name="rt", bufs=1) as rt,
        ):
            # ---- static p1 weights [96, (hl q n)] one DMA
            wp1t = wp1pool.tile([96, 2 * 3 * CB], F16, name="wp1t", tag="wp1t")
            nc.sync.dma_start(out=wp1t[:].rearrange("p (h q n) -> p h q n", h=2, q=3),
                              in_=wp1_d.ap())
            wp1v = wp1t[:].rearrange("p (h q n) -> p h q n", h=2, q=3)

            # ---- resident hi weights [9, A*CB] f16 (slices loaded lazily)
            wvh = wp1pool.tile([9, A * CB], F16, name="wvh", tag="wvh")
            wvh3 = wvh[:].rearrange("q (a n) -> q a n", a=A)
            wvh_loaded = [False] * 4

            def ensure_wvh(qw):
                if not wvh_loaded[qw]:
                    asl = slice(qw * 8, (qw + 1) * 8)
                    nc.sync.dma_start(
                        out=wvh3[:, asl, :],
                        in_=wvh_d.ap()[:, asl, :])
                    wvh_loaded[qw] = True

            # ---- shared routing scratch (aliased: delta1 f32 views live in
            # scrA/scrB/tr2/tr3 whose f16 users are temporally disjoint)
            scrA = rt.tile([P, 16384], F16, tag="scrA")   # prod16 | pd1v/pd1g
            scrB = rt.tile([P, 8192], F16, tag="scrB")    # tree1  | t1d1v/g | praw/t3d1
            tr2 = rt.tile([P, 4096], F16, tag="tr2")      # tree2  | t2d1v/g | elog
            tr3 = rt.tile([P, 2048], F16, tag="tr3")      # tree3
            tr4 = rt.tile([P, 1024], F16, tag="tr4")      # tree4
            pcur1t = rt.tile([P, CB], F32, tag="pcur1t")
            SCR = {
                "prod16": scrA[:],
                "tree1": scrB[:],
                "tree2": tr2[:],
                "tree3": tr3[:],
                "tree4": tr4[:],
                "sqp2": tr4[:, 0:1024].bitcast(F32),
                "pd1v": scrA[:, 0:4096].bitcast(F32),
                "pd1g": scrA[:, 4096:8192].bitcast(F32),
                "t1d1v": scrB[:, 0:2048].bitcast(F32),
                "t1d1g": scrB[:, 2048:4096].bitcast(F32),
                "t2d1v": tr2[:, 0:1024].bitcast(F32),
                "t2d1g": tr2[:, 1024:2048].bitcast(F32),
                "t3d1v": scrB[:, 5120:5632].bitcast(F32),
                "t3d1g": scrB[:, 5632:6144].bitcast(F32),
                "elog": tr2[:, 2048:4096].bitcast(F32),
                "praw": scrB[:, 4096:5120].bitcast(F32),
                "pcur1": pcur1t[:],
            }

            def emit_votes(k):
                """DMA + PE voting + Act drains for chunk k."""
                ksl = slice(k * NCHUNK, (k + 1) * NCHUNK)
                pp1t = winpool.tile([96, 2 * 3 * NCHUNK], F16, name="pp1t", tag="pp1t", bufs=1)
                nc.sync.dma_start(
                    out=pp1t[:].rearrange("p (h q n) -> p h q n", h=2, q=3),
                    in_=pp1_d.ap()[:, :, :, ksl])
                pp1v = pp1t[:].rearrange("p (h q n) -> p h q n", h=2, q=3)
                # p1 matmuls into a shared psv-slot (uses first bank only)
                p1t = psv.tile([P, AG * CB], F32, tag="vps")
                p1ps = p1t[:, :CB]
                for q in range(3):
                    nc.tensor.matmul(out=p1ps, lhsT=pp1v[:, 0, q, :], rhs=wp1v[:, 0, q, :],
                                     start=(q == 0), stop=False)
                    nc.tensor.matmul(out=p1ps, lhsT=pp1v[:, 0, q, :], rhs=wp1v[:, 1, q, :],
                                     start=False, stop=False)
                    nc.tensor.matmul(out=p1ps, lhsT=pp1v[:, 1, q, :], rhs=wp1v[:, 0, q, :],
                                     start=False, stop=(q == 2))
                p1raw = rt.tile([P, CB], F32, tag="p1raw")
                nc.scalar.mul(out=p1raw[:], in_=p1ps, mul=1.0 / A)

                v16b = v16pool.tile([P, CB * A], F16, tag="v16b")   # (c,b,a)
                v16b4 = v16b[:].rearrange("p (c b a) -> p c b a", c=C, b=B)
                v32gs = []
                for g in range(NG):
                    w0 = g * AG
                    pkt = winpool.tile([9, AG * 2 * NCHUNK], F16, name="pk", tag="pk")
                    nc.sync.dma_start(
                        out=pkt[:].rearrange("q (a h n) -> q a h n", a=AG, h=2),
                        in_=pat2_d.ap()[:, w0:w0 + AG, :, ksl])
                    pkv = pkt[:].rearrange("q (a h n) -> q a h n", a=AG, h=2)
                    if True:
                        wlt = winpool.tile([9, AG * CB], F16, name="wgl", tag="wgl")
                        nc.sync.dma_start(
                            out=wlt[:].rearrange("q (a n) -> q a n", a=AG),
                            in_=wvl_d.ap()[:, w0:w0 + AG, :])
                        wl3 = wlt[:].rearrange("q (a n) -> q a n", a=AG)
                    ensure_wvh(g // 2)
                    vps = psv.tile([P, AG * CB], F32, tag="vps")
                    for ai in range(AG):
                        a_glob = g * AG + ai
                        aw = ai
                        osl = vps[:, ai * CB:(ai + 1) * CB]
                        nc.tensor.matmul(out=osl, lhsT=pkv[:, ai, 0, :], rhs=wvh3[:, a_glob, :],
                                         start=True, stop=False)
                        nc.tensor.matmul(out=osl, lhsT=pkv[:, ai, 0, :], rhs=wl3[:, aw, :],
                                         start=False, stop=False)
                        nc.tensor.matmul(out=osl, lhsT=pkv[:, ai, 1, :], rhs=wvh3[:, a_glob, :],
                                         start=False, stop=True)
                    for ai in range(AG):
                        a_glob = g * AG + ai
                        nc.scalar.copy(
                            out=v16b4[:, :, :, a_glob],
                            in_=vps[:, ai * CB:(ai + 1) * CB].rearrange("p (c b) -> p c b", c=C))
                    v32g = v32pool.tile([P, AG * CB], F32, tag="v32g")
                    nc.scalar.copy(out=v32g[:], in_=vps[:])
                    v32gs.append(v32g)
                return p1raw, v16b4, v32gs

            def squash(praw_cb, pcur_tag, sq_tag):
                p2 = SCR["sqp2"]
                nc.gpsimd.tensor_tensor(out=p2, in0=praw_cb, in1=praw_cb, op=ALU.mult)
                s = rt.tile([P, B], F32, tag=f"{sq_tag}s")
                nc.vector.tensor_reduce(
                    out=s[:], in_=p2.rearrange("p (c b) -> p b c", c=C),
                    axis=mybir.AxisListType.X, op=ALU.add)
                nc.vector.tensor_scalar_add(out=s[:], in0=s[:], scalar1=EPS)  # s = sq
                nrm = rt.tile([P, B], F32, tag=f"{sq_tag}n")
                nc.scalar.activation(out=nrm[:], in_=s[:], func=ACTF.Sqrt)
                fac = rt.tile([P, B], F32, tag=f"{sq_tag}f")
                nc.vector.scalar_tensor_tensor(out=fac[:], in0=s[:], scalar=1.0,
                                               in1=nrm[:], op0=ALU.add, op1=ALU.mult)
                nc.vector.tensor_scalar_add(out=fac[:], in0=fac[:], scalar1=EPS)
                nc.vector.reciprocal(out=fac[:], in_=fac[:])
                nc.vector.tensor_tensor(out=fac[:], in0=s[:], in1=fac[:], op=ALU.mult)
                if pcur_tag == "pcur1":
                    pcur_ap = SCR["pcur1"]
                else:
                    pcur_t = rt.tile([P, CB], F32, tag=pcur_tag, name=pcur_tag)
                    pcur_ap = pcur_t[:]
                nc.vector.tensor_tensor(
                    out=pcur_ap.rearrange("p (c b) -> p c b", c=C),
                    in0=praw_cb.rearrange("p (c b) -> p c b", c=C),
                    in1=fac[:].unsqueeze(1).to_broadcast([P, C, B]),
                    op=ALU.mult)
                return pcur_ap

            def emit_delta1(v32gs, pcur1, logits1, groups):
                """logits1 (a,b) f32 = sum_c V*P1 (groups subset)."""
                l1v = logits1[:].rearrange("p (a b) -> p a b", a=A)
                p1b = pcur1.rearrange("p (c b) -> p c b", c=C) \
                    .unsqueeze(1).to_broadcast([P, AG, C, B])
                for g in groups:
                    # Pool takes the earliest groups (drained first) so it can
                    # start while DVE is still in the routing chain
                    eng = nc.gpsimd if g < (NG - GSPL) else nc.vector
                    sfx = "g" if g < (NG - GSPL) else "v"
                    v32g4 = v32gs[g][:].rearrange("p (a c b) -> p a c b", a=AG, c=C)
                    pd4 = SCR[f"pd1{sfx}"].rearrange("p (a c b) -> p a c b", a=AG, c=C)
                    eng.tensor_tensor(out=pd4, in0=v32g4, in1=p1b, op=ALU.mult)
                    t14 = SCR[f"t1d1{sfx}"].rearrange("p (a c b) -> p a c b", a=AG, c=8)
                    eng.tensor_tensor(out=t14, in0=pd4[:, :, 0:8, :], in1=pd4[:, :, 8:16, :], op=ALU.add)
                    t24 = SCR[f"t2d1{sfx}"].rearrange("p (a c b) -> p a c b", a=AG, c=4)
                    eng.tensor_tensor(out=t24, in0=t14[:, :, 0:4, :], in1=t14[:, :, 4:8, :], op=ALU.add)
                    t34 = SCR[f"t3d1{sfx}"].rearrange("p (a c b) -> p a c b", a=AG, c=2)
                    eng.tensor_tensor(out=t34, in0=t24[:, :, 0:2, :], in1=t24[:, :, 2:4, :], op=ALU.add)
                    eng.tensor_tensor(out=l1v[:, g * AG:(g + 1) * AG, :],
                                      in0=t34[:, :, 0, :], in1=t34[:, :, 1, :], op=ALU.add)

            def softmax(logits, layout, lsm16):
                elog = SCR["elog"]
                nc.scalar.activation(out=elog, in_=logits[:], func=ACTF.Exp)
                ssum = rt.tile([P, B], F32, tag="sqs")
                if layout == "ab":
                    ev = elog.rearrange("p (a b) -> p a b", a=A)
                    nc.vector.tensor_reduce(out=ssum[:], in_=ev.rearrange("p a b -> p b a"),
                                            axis=mybir.AxisListType.X, op=ALU.add)
                    nc.vector.reciprocal(out=ssum[:], in_=ssum[:])
                    nc.vector.tensor_tensor(
                        out=lsm16[:].rearrange("p (b a) -> p b a", b=B).rearrange("p b a -> p a b"),
                        in0=ev,
                        in1=ssum[:].unsqueeze(1).to_broadcast([P, A, B]),
                        op=ALU.mult)
                else:
                    ev = elog.rearrange("p (b a) -> p b a", b=B)
                    nc.vector.tensor_reduce(out=ssum[:], in_=ev,
                                            axis=mybir.AxisListType.X, op=ALU.add)
                    nc.vector.reciprocal(out=ssum[:], in_=ssum[:])
                    nc.vector.tensor_tensor(
                        out=lsm16[:].rearrange("p (b a) -> p b a", b=B),
                        in0=ev,
                        in1=ssum[:].unsqueeze(2).to_broadcast([P, B, A]),
                        op=ALU.mult)

            def emit_preds(v16b4, lsm16, praw):
                """praw (c,b) f32 = sum_a lsm*V ; DVE c[0:CSPL], Pool rest."""
                pr4 = SCR["prod16"].rearrange("p (c b a) -> p c b a", c=C, b=B)
                t14 = SCR["tree1"].rearrange("p (c b a) -> p c b a", c=C, b=B)
                t24 = SCR["tree2"].rearrange("p (c b a) -> p c b a", c=C, b=B)
                t34 = SCR["tree3"].rearrange("p (c b a) -> p c b a", c=C, b=B)
                t44 = SCR["tree4"].rearrange("p (c b a) -> p c b a", c=C, b=B)
                prv = praw.rearrange("p (c b) -> p c b", c=C)
                for eng, cs in ((nc.vector, slice(0, CSPL)), (nc.gpsimd, slice(CSPL, C))):
                    ncs = cs.stop - cs.start
                    lbb = lsm16[:].rearrange("p (b a) -> p b a", b=B) \
                        .unsqueeze(1).to_broadcast([P, ncs, B, A])
                    eng.tensor_tensor(out=pr4[:, cs, :, :], in0=v16b4[:, cs, :, :],
                                      in1=lbb, op=ALU.mult)
                    eng.tensor_tensor(out=t14[:, cs, :, :], in0=pr4[:, cs, :, 0:16],
                                      in1=pr4[:, cs, :, 16:32], op=ALU.add)
                    eng.tensor_tensor(out=t24[:, cs, :, :], in0=t14[:, cs, :, 0:8],
                                      in1=t14[:, cs, :, 8:16], op=ALU.add)
                    eng.tensor_tensor(out=t34[:, cs, :, :], in0=t24[:, cs, :, 0:4],
                                      in1=t24[:, cs, :, 4:8], op=ALU.add)
                    eng.tensor_tensor(out=t44[:, cs, :, :], in0=t34[:, cs, :, 0:2],
                                      in1=t34[:, cs, :, 2:4], op=ALU.add)
                    eng.tensor_tensor(out=prv[:, cs, :], in0=t44[:, cs, :, 0],
                                      in1=t44[:, cs, :, 1], op=ALU.add)

            def emit_delta2(v16b4, pcur2, lsm16, logits2):
                """logits2 (b,a) f32 = sum_c V*P2 + lsm2 ; split by b."""
                t44 = SCR["tree4"].rearrange("p (b a) -> p b a", b=B)
                t4 = SCR["tree4"]
                pr4 = SCR["prod16"].rearrange("p (c b a) -> p c b a", c=C, b=B)
                t14 = SCR["tree1"].rearrange("p (c b a) -> p c b a", c=8, b=B)
                t24 = SCR["tree2"].rearrange("p (c b a) -> p c b a", c=4, b=B)
                t34 = SCR["tree3"].rearrange("p (c b a) -> p c b a", c=2, b=B)
                p2v = pcur2.rearrange("p (c b) -> p c b", c=C)
                # Pool takes most of the (broadcast-penalized) mult in sub-slices;
                # DVE trees chase slice-by-slice.
                msl = [slice(0, 5), slice(5, 14), slice(14, 23), slice(23, 32)]
                for i, bs in enumerate(msl):
                    eng = nc.vector if i == 0 else nc.gpsimd
                    nbs = bs.stop - bs.start
                    p2b = p2v[:, :, bs].unsqueeze(3).to_broadcast([P, C, nbs, A])
                    eng.tensor_tensor(out=pr4[:, :, bs, :], in0=v16b4[:, :, bs, :],
                                      in1=p2b, op=ALU.mult)
                for bs in msl:
                    nc.vector.tensor_tensor(out=t14[:, :, bs, :], in0=pr4[:, 0:8, bs, :],
                                            in1=pr4[:, 8:16, bs, :], op=ALU.add)
                    nc.vector.tensor_tensor(out=t24[:, :, bs, :], in0=t14[:, 0:4, bs, :],
                                            in1=t14[:, 4:8, bs, :], op=ALU.add)
                    nc.vector.tensor_tensor(out=t34[:, :, bs, :], in0=t24[:, 0:2, bs, :],
                                            in1=t24[:, 2:4, bs, :], op=ALU.add)
                    nc.vector.tensor_tensor(out=t44[:, bs, :], in0=t34[:, 0, bs, :],
                                            in1=t34[:, 1, bs, :], op=ALU.add)
                nc.vector.tensor_tensor(out=logits2[:], in0=t4, in1=lsm16[:], op=ALU.add)

            # ================== main schedule ==================
            p1raw0, v16b40, v32gs0 = emit_votes(0)
            pcur1_0 = squash(p1raw0[:], "pcur1", "sq")
            logits = rt.tile([P, A * B], F32, tag="logits")
            if num_routes >= 2:
                emit_delta1(v32gs0, pcur1_0, logits, range(NG))
                nc.vector.tensor_scalar_add(out=logits[:], in0=logits[:], scalar1=1.0 / A)
            state = {"v16b4": v16b40, "pcur1": pcur1_0}

            for k in range(NCH):
                v16b4 = state["v16b4"]
                pcur1 = state["pcur1"]
                nxt = k + 1 < NCH
                p1raw_n = pcur1_n = v16b4_n = v32gs_n = None
                votes_emitted = False
                sq1_done = False

                def emit_votes_once():
                    nonlocal votes_emitted, p1raw_n, v16b4_n, v32gs_n
                    if nxt and not votes_emitted:
                        p1raw_n, v16b4_n, v32gs_n = emit_votes(k + 1)
                        votes_emitted = True

                def inject(groups, first=False):
                    nonlocal sq1_done, pcur1_n
                    if not nxt or num_routes < 2:
                        return
                    emit_votes_once()
                    if not sq1_done:
                        pcur1_n = squash(p1raw_n[:], "pcur1", "sq")
                        sq1_done = True
                    emit_delta1(v32gs_n, pcur1_n, logits, groups)

                if num_routes == 1:
                    pcur_fin = pcur1
                    inject(range(NG))
                else:
                    layout = "ab"
                    lsm16 = rt.tile([P, B * A], F16, tag="lsm16")
                    pcur_fin = None
                    for it in range(2, num_routes + 1):
                        softmax(logits, layout, lsm16)
                        if it == 2:
                            emit_votes_once()   # after sm2's exp gets Act priority
                        praw = SCR["praw"]
                        emit_preds(v16b4, lsm16, praw)
                        last = (it == num_routes)
                        if last:
                            inject(range(0, 3))
                        pcur = squash(praw, "pcurI", "sq")
                        if not last:
                            emit_delta2(v16b4, pcur, lsm16, logits)
                            layout = "ba"
                        else:
                            pcur_fin = pcur
                        if last:
                            inject(range(3, 6))

                nc.sync.dma_start(out=out_d.ap()[k * NCHUNK:(k + 1) * NCHUNK, :],
                                  in_=pcur_fin)

                if nxt:
                    inject(range(6, NG))
                    if num_routes >= 2:
                        nc.vector.tensor_scalar_add(out=logits[:], in0=logits[:],
                                                    scalar1=1.0 / A)
                    state = {"v16b4": v16b4_n, "pcur1": pcur1_n}

    nc.compile()
    return nc


def _prep(x, weights):
    xp = np.zeros((A, H + 2, W + 2), dtype=np.float32)
    xp[:, 1:-1, 1:-1] = x
    wvf = np.ascontiguousarray(weights.reshape(9, A, C * B))  # (c,b): idx c*B+b
    wv_h = wvf.astype(np.float16)
    wv_l = (wvf - wv_h.astype(np.float32)).astype(np.float16)
    # wp1 packed [96, 2, 3, CB]: row = (pq_local*32 + a) for q-block
    wp1 = np.empty((96, 2, 3, CB), dtype=np.float16)
    for q in range(3):
        wp1[:, 0, q, :] = wv_h[3 * q:3 * q + 3].reshape(96, CB)
        wp1[:, 1, q, :] = wv_l[3 * q:3 * q + 3].reshape(96, CB)
    in_maps = []
    for core in range(NCORES):
        r0 = core * ROWS
        pat = np.empty((9, A, ROWS, W), dtype=np.float32)
        for dp in range(3):
            for dq in range(3):
                pat[dp * 3 + dq] = xp[:, r0 + dp:r0 + dp + ROWS, dq:dq + W]
        patf = np.ascontiguousarray(pat.reshape(9, A, NPOS))
        pat_h = patf.astype(np.float16)
        pat_l = (patf - pat_h.astype(np.float32)).astype(np.float16)
        pat2 = np.stack([pat_h, pat_l], axis=2)  # [9, A, 2, NPOS]
        pp1 = np.empty((96, 2, 3, NPOS), dtype=np.float16)
        for q in range(3):
            pp1[:, 0, q, :] = pat_h[3 * q:3 * q + 3].reshape(96, NPOS)
            pp1[:, 1, q, :] = pat_l[3 * q:3 * q + 3].reshape(96, NPOS)
        in_maps.append({"pat2": np.ascontiguousarray(pat2), "wvh": wv_h, "wvl": wv_l,
                        "pp1": pp1, "wp1": wp1})
    return in_maps


def kernel(x=None, weights=None, num_routes=3, **kw):
    x = np.asarray(x, dtype=np.float32)
    weights = np.asarray(weights, dtype=np.float32)
    nr = int(num_routes)
    if nr not in _CACHE:
        _CACHE[nr] = _build_nc(nr)
    nc = _CACHE[nr]
    in_maps = _prep(x, weights)
    res = bass_utils.run_bass_kernel_spmd(nc, in_maps, core_ids=list(range(NCORES)))
    out = np.empty((B, C, H, W), dtype=np.float32)
    for core in range(NCORES):
        o = np.asarray(res.results[core]["out"]).reshape(ROWS, W, C, B)
        out[:, :, core * ROWS:(core + 1) * ROWS, :] = o.transpose(3, 2, 0, 1)
    return out


def profile_once(inputs):
    from concourse import bass_interp
    nr = int(inputs.get("num_routes", 3))
    if nr not in _CACHE:
        _CACHE[nr] = _build_nc(nr)
    sim = bass_interp.CoreSim(_CACHE[nr], no_exec=True, ignore_data_errors=True,
                              publish_trace=False)
    sim.simulate()
    return int(sim.time)
